# revision 53
# baseline (speedup 1.0000x reference)
"""DeepWalk community-pooling kernel for 8 trn2 NeuronCores (v2).

Pipeline (per core, SPMD identical program, per-core data):
  host: sort extended rows (N + multi duplicates) by community, pad each
        community to a multiple of 8 rows, deal communities per size-class
        round-robin onto 48 (core, lane) slots (6 lanes/core) so every
        slot has an identical class profile.
  device, per 512-column "pb block" (512 stream indices x 6 lanes = 3072
  rows):
    mmA  : ds3^T 3-lane-packed [60,1024] x wa3 -> pa [120,1024] psum
    hx   : ACT relu+bias -> hx3 bf16 [120,1024]
    mm_h : wh^T x hx3 chunks -> pb[0:60] / pb[64:124] (accumulate)
    mm_xw: wxw^T x xf6 [126,512] 6-lane-packed -> pb (x-contribution +
           pad-flag), one matmul at 6-row/col density
    reluB: (pb + b_feat) relu -> y bf16 [124,512]  (ACT or DVE, balanced)
    sum  : DVE TT-tree radix-8 (2x bf16 mode) -> g1s
    max  : GPSIMD TT-tree radix-8 -> g1m
    lvl2 : per size-class tensor_reduce over k groups -> g2s (f32), g2m
  tail:  mean = g2s * recip (host-provided reciprocals), final GEMM
         relu(W_out^T [mean; max] + b_out) -> out [96, c6p]
  host: gather per-lane outputs back to the global community order.
"""

import sys

import numpy as np

sys.path.insert(0, "/opt/trn_rl_repo")

import ml_dtypes  # noqa: E402

BF16 = ml_dtypes.bfloat16
FP8 = ml_dtypes.float8_e4m3fn

N = 2_000_000
M = 500_000
C = 50_000
D_OUT = 16
N_CORES = 8
N_LANES = 6  # per core
SLOTS = N_CORES * N_LANES
BLK = 512  # pb columns per block
FLAG_PAD = -32768.0
W3_DMA = 8192  # ds3 cols per input DMA tile (= 4096 stream idx)
LANE_OFF = [0, 20, 40, 64, 84, 104]  # partition offset of each lane block
RELUB_ACT = frozenset({1, 4, 6})  # b % 8 in this set -> reluB on ACT
N_WARMUP = 17  # back-to-back warm-up matmuls to flip the PE HAM to 2.4 GHz


# ----------------------------------------------------------------------------
# Host-side planning
# ----------------------------------------------------------------------------

def _plan(community, multi_community_index, multi_community_nodes):
    """Sort/pad/shard rows. Returns per-core row sources + static layout."""
    seg = np.concatenate([community, multi_community_index]).astype(np.int64)
    src = np.concatenate(
        [np.arange(N, dtype=np.int64), multi_community_nodes.astype(np.int64)]
    )

    counts = np.bincount(seg, minlength=C)
    kcls = np.maximum((counts + 7) // 8, 1).astype(np.int64)  # class = #groups
    assert kcls.max() <= 64, f"community too large: {counts.max()} rows"

    order = np.argsort(seg, kind="stable")
    src_sorted = src[order]
    starts = np.zeros(C + 1, dtype=np.int64)
    np.cumsum(counts, out=starts[1:])

    # communities per class, dealt round-robin to 48 (core,lane) slots.
    # Classes are laid out largest-first so the level-2 reductions of the
    # big classes complete early and the final GEMM pipelines with the
    # main loop.
    classes = np.unique(kcls)[::-1]
    slot_comms = [[[] for _ in range(N_LANES)] for _ in range(N_CORES)]
    n48 = {}  # class k -> communities per slot
    for k in classes:
        comms = np.nonzero(kcls == k)[0]
        n48[int(k)] = (len(comms) + SLOTS - 1) // SLOTS
        for i, g in enumerate(comms):
            s = i % SLOTS
            slot_comms[s // N_LANES][s % N_LANES].append(int(g))
    classes = [int(k) for k in classes]

    # per-lane group/community layout (identical across all cores/lanes)
    lane_groups = sum(n48[k] * k for k in classes)
    c6 = sum(n48[k] for k in classes)  # community slots per lane
    c6p = ((c6 + BLK - 1) // BLK) * BLK
    lane_rows = lane_groups * 8
    lane_len = ((lane_rows + BLK - 1) // BLK) * BLK

    # class offsets (group units and community-slot units)
    a_k, c_k, ga, ca = {}, {}, 0, 0
    for k in classes:
        a_k[k] = ga
        c_k[k] = ca
        ga += n48[k] * k
        ca += n48[k]

    # per (core,lane): row source indices (-1 = padding), per-slot counts
    core_data = []
    for ci in range(N_CORES):
        lane_src = np.full((N_LANES, lane_len), -1, dtype=np.int64)
        lane_flag = np.full((N_LANES, lane_len), FLAG_PAD, dtype=np.float32)
        slot_count = np.zeros((N_LANES, c6p), dtype=np.int64)
        slot_comm = np.full((N_LANES, c6p), -1, dtype=np.int64)
        for lj in range(N_LANES):
            comms = slot_comms[ci][lj]
            by_k = {k: [] for k in classes}
            for g in comms:
                by_k[int(kcls[g])].append(g)
            pos = 0
            for k in classes:
                lst = by_k[k]
                for i in range(n48[k]):
                    slot = c_k[k] + i
                    if i < len(lst):
                        g = lst[i]
                        cnt = int(counts[g])
                        s0 = starts[g]
                        lane_src[lj, pos : pos + cnt] = src_sorted[s0 : s0 + cnt]
                        lane_flag[lj, pos : pos + cnt] = 0.0
                        slot_count[lj, slot] = cnt
                        slot_comm[lj, slot] = g
                    pos += 8 * k
            assert pos == lane_rows
        core_data.append((lane_src, lane_flag, slot_count, slot_comm))

    layout = dict(
        classes=classes, n48=n48, a_k=a_k, c_k=c_k,
        c6=c6, c6p=c6p, lane_len=lane_len, lane_groups=lane_groups,
    )
    return core_data, layout


def _build_core_inputs(core_dat, layout, x, dataset_x):
    """Build the DRAM images for one core."""
    lane_src, lane_flag, slot_count, _ = core_dat
    lane_len = layout["lane_len"]
    c6p = layout["c6p"]
    nblk = lane_len // BLK
    F3 = 2 * lane_len
    F6 = lane_len

    idx = np.maximum(lane_src, 0)

    # ds3 [60, F3]: col 1024b+512t+j holds lanes {3t,3t+1,3t+2} at stream
    # index 512b+j; lane 3t+m occupies partitions 20m..20m+20. fp8: the
    # demo/purch MLP path tolerates e4m3 (verified ~0.004 end-to-end).
    arr = dataset_x[idx].astype(FP8)               # [6, lane_len, 20]
    arrv = arr.reshape(2, 3, nblk, BLK, 20)        # [t, m, b, j, f]
    ds3 = np.zeros((64, F3), dtype=FP8)
    ds3[0:60] = arrv.transpose(1, 4, 2, 0, 3).reshape(60, F3)

    # xf6 [126, F6]: col i holds all 6 lanes at stream index i;
    # lane l occupies partitions 21l..21l+20 (+ flag channel at 21l+20).
    xv = x[idx].astype(BF16)                       # [6, lane_len, 20]
    xf6 = np.zeros((128, F6), dtype=BF16)
    for l in range(N_LANES):
        xf6[21 * l : 21 * l + 20] = xv[l].T
        xf6[21 * l + 20] = lane_flag[l].astype(BF16)

    recip = np.ones((124, c6p), dtype=np.float32)
    for l in range(N_LANES):
        r = 1.0 / np.maximum(slot_count[l], 1).astype(np.float32)
        off = LANE_OFF[l]
        recip[off : off + 20, :] = r[None, :]

    return dict(ds3=ds3, xf6=xf6, recip=recip)


def _build_shared_inputs(params):
    (W_demo, b_demo, W_purch, b_purch, W_feat, b_feat, W_out, b_out) = params

    # mmA stationary [128, 120]: 3 lanes; lane t ds feats at partitions
    # 20t..20t+20 -> h (demo|purch) at out cols 40t..40t+40. All matmul
    # contracts are zero-padded to the full 128 rows: the PE HAM activity
    # monitor only un-throttles the clock gate (1.2 -> 2.4 GHz) for
    # full-height operands.
    wa3 = np.zeros((128, 120), dtype=FP8)
    for t in range(3):
        wa3[20 * t : 20 * t + 8, 40 * t : 40 * t + 20] = W_demo
        wa3[20 * t + 8 : 20 * t + 20, 40 * t + 20 : 40 * t + 40] = W_purch

    ba3 = np.zeros((120, 1), dtype=np.float32)
    for t in range(3):
        ba3[40 * t : 40 * t + 20, 0] = b_demo
        ba3[40 * t + 20 : 40 * t + 40, 0] = b_purch

    # mm_h stationary [128, 60]: lane t h-feats at 40t..40t+40 -> y cols
    # 20t..20t+20 (chunk A lanes 0-2 at pb[0:60], chunk B lanes 3-5 at
    # pb[64:124])
    wh = np.zeros((128, 60), dtype=BF16)
    for t in range(3):
        wh[40 * t : 40 * t + 40, 20 * t : 20 * t + 20] = W_feat[0:40]

    # mm_xw stationary [128, 124]: 6-lane-packed x -> x-part of y, plus the
    # pad flag channel -> -32768 on that lane's 20 y cols
    wxw = np.zeros((128, 124), dtype=BF16)
    for l in range(N_LANES):
        off = LANE_OFF[l]
        wxw[21 * l : 21 * l + 20, off : off + 20] = W_feat[40:60]
        wxw[21 * l + 20, off : off + 20] = 1.0

    bb6 = np.zeros((124, 1), dtype=np.float32)
    for l in range(N_LANES):
        off = LANE_OFF[l]
        bb6[off : off + 20, 0] = b_feat

    # final GEMM stationaries [124, 96]: lane l mean/max rows -> out cols
    # 16l..16l+16
    woutm = np.zeros((124, 96), dtype=BF16)
    woutx = np.zeros((124, 96), dtype=BF16)
    for l in range(N_LANES):
        off = LANE_OFF[l]
        woutm[off : off + 20, 16 * l : 16 * l + 16] = W_out[0:20]
        woutx[off : off + 20, 16 * l : 16 * l + 16] = W_out[20:40]

    bo6 = np.zeros((96, 1), dtype=np.float32)
    for l in range(N_LANES):
        bo6[16 * l : 16 * l + 16, 0] = b_out

    return dict(wa3=wa3, ba3=ba3, wh=wh, wxw=wxw, bb6=bb6,
                woutm=woutm, woutx=woutx, bo6=bo6)


# ----------------------------------------------------------------------------
# Device kernel
# ----------------------------------------------------------------------------

def _build_nc(layout):
    import concourse.bacc as bacc
    import concourse.mybir as mybir
    from concourse import tile

    f32 = mybir.dt.float32
    bf16 = mybir.dt.bfloat16
    f8 = mybir.dt.float8e4

    lane_len = layout["lane_len"]
    c6p = layout["c6p"]
    nblk = lane_len // BLK
    F3 = 2 * lane_len
    F6 = lane_len
    G1 = nblk * 64  # lvl-1 group columns (64 per block)
    classes = layout["classes"]
    n48 = layout["n48"]
    a_k = layout["a_k"]
    c_k = layout["c_k"]

    nc = bacc.Bacc("TRN2", target_bir_lowering=False, debug=False)

    dt_map = dict(ds3=f8, xf6=bf16, recip=f32, wa3=f8, wh=bf16, wxw=bf16,
                  woutm=bf16, woutx=bf16, ba3=f32, bb6=f32, bo6=f32)
    shapes = dict(ds3=[64, F3], xf6=[128, F6], recip=[124, c6p],
                  wa3=[128, 120], wh=[128, 60], wxw=[128, 124],
                  woutm=[124, 96], woutx=[124, 96],
                  ba3=[120, 1], bb6=[124, 1], bo6=[96, 1])
    dram = {
        name: nc.declare_dram_parameter(name, shapes[name], dt_map[name],
                                        isOutput=False)
        for name in shapes
    }
    out_d = nc.declare_dram_parameter("out", [96, c6p], f32, isOutput=True)

    AX = mybir.AxisListType.X
    OP = mybir.AluOpType
    RELU = mybir.ActivationFunctionType.Relu

    with tile.TileContext(nc) as tc:
        with (
            tc.tile_pool(name="wpool", bufs=1) as wpool,
            tc.tile_pool(name="g", bufs=1) as gpool,
            tc.tile_pool(name="ds3p", bufs=2) as ds3p,
            tc.tile_pool(name="xf6p", bufs=2) as xf6p,
            tc.tile_pool(name="hxp", bufs=3) as hxp,
            tc.tile_pool(name="yp", bufs=3) as yp,
            tc.tile_pool(name="t1p", bufs=2) as t1p,
            tc.tile_pool(name="t2p", bufs=4) as t2p,
            tc.tile_pool(name="m1p", bufs=2) as m1p,
            tc.tile_pool(name="pa", bufs=3, space="PSUM") as pap,
            tc.tile_pool(name="pb", bufs=2, space="PSUM") as pbp,
            tc.tile_pool(name="outp", bufs=1) as outp,
        ):
            wa3_t = wpool.tile([128, 120], f8, tag="wa3")
            wh_t = wpool.tile([128, 60], bf16, tag="wh")
            wxw_t = wpool.tile([128, 124], bf16, tag="wxw")
            woutm_t = wpool.tile([124, 96], bf16, tag="woutm")
            woutx_t = wpool.tile([124, 96], bf16, tag="woutx")
            ba3_t = wpool.tile([120, 1], f32, tag="ba3")
            bb6_t = wpool.tile([124, 1], f32, tag="bb6")
            bo6_t = wpool.tile([96, 1], f32, tag="bo6")
            recip_t = wpool.tile([124, c6p], f32, tag="recip")

            # fixed hand-rotated input/hx tiles, zero-padded to 128 rows so
            # every matmul streams a full-height rhs (HAM activity)
            ds3_ts = [wpool.tile([128, W3_DMA], f8, tag=f"ds3{i}",
                                 name=f"ds3{i}") for i in range(3)]
            xf6_ts = [wpool.tile([128, W3_DMA // 2], bf16, tag=f"xf6{i}",
                                 name=f"xf6{i}") for i in range(3)]
            hx_ts = [wpool.tile([128, 1024], bf16, tag=f"hx{i}",
                                name=f"hx{i}") for i in range(3)]
            wtmp = wpool.tile([128, 512], bf16, tag="wtmp")
            nc.gpsimd.memset(wtmp[:, :], 0.0)
            # the first 4096 cols gate chunk 0's matmuls; the rest of tile 0
            # is only read from chunk 3 on, so zero it off the critical path
            nc.gpsimd.memset(ds3_ts[0][64:128, 0:4096], 0.0)
            nc.gpsimd.memset(ds3_ts[0][64:128, 4096:W3_DMA], 0.0)
            nc.gpsimd.memset(ds3_ts[1][64:128, :], 0.0)
            nc.vector.memset(ds3_ts[2][64:128, :], 0.0)
            nc.scalar.memzero(hx_ts[0][96:128, :])
            nc.scalar.memzero(hx_ts[1][96:128, :])
            nc.scalar.memzero(hx_ts[2][96:128, :])

            # input chunk schedule: two small leading chunks cut the
            # time-to-first-block; input triggers precede the bulky weight
            # transfers on the sync queue.
            chunk_list = []
            o3 = 0
            while o3 < F3:
                w3 = min(4096 if o3 < 8192 else W3_DMA, F3 - o3)
                chunk_list.append((o3, w3))
                o3 += w3

            def trigger_chunk(di):
                o3, w3 = chunk_list[di]
                ds3_t = ds3_ts[di % 3]
                xf6_t = xf6_ts[di % 3]
                nc.sync.dma_start(out=ds3_t[0:64, :w3],
                                  in_=dram["ds3"][:, o3 : o3 + w3])
                nc.sync.dma_start(out=xf6_t[0:128, : w3 // 2],
                                  in_=dram["xf6"][:, o3 // 2 : (o3 + w3) // 2])

            trigger_chunk(0)
            for name, t in [("wa3", wa3_t), ("ba3", ba3_t)]:
                nc.sync.dma_start(out=t[:], in_=dram[name][:])

            # PE HAM warm-up, gated on the first input chunk (via the copy
            # below): back-to-back full-128x128 matmuls give the activity
            # monitor a fully-busy 4096-cycle window, flipping the PE clock
            # gate from its default 1.2 GHz to 2.4 GHz; the main loop's own
            # dense stream keeps it warm from there.
            nc.vector.tensor_copy(out=wtmp[0:1, 0:1], in_=ds3_ts[0][0:1, 0:1])
            pw0 = pbp.tile([128, BLK], f32, tag="pb")
            for _ in range(N_WARMUP):
                nc.tensor.matmul(pw0[:, :], lhsT=wtmp[:, 0:128],
                                 rhs=wtmp[:, :], start=True, stop=True)

            for name, t in [("wh", wh_t), ("wxw", wxw_t), ("bb6", bb6_t)]:
                nc.sync.dma_start(out=t[:], in_=dram[name][:])
            trigger_chunk(1)
            for name, t in [("woutm", woutm_t), ("woutx", woutx_t),
                            ("bo6", bo6_t)]:
                nc.sync.dma_start(out=t[:], in_=dram[name][:])
            trigger_chunk(2)
            nc.sync.dma_start(out=recip_t[:], in_=dram["recip"][:])

            g1s = gpool.tile([124, G1], bf16, tag="g1s")
            g1m = gpool.tile([124, G1], bf16, tag="g1m")
            g2s = gpool.tile([124, c6p], f32, tag="g2s")
            g2m = gpool.tile([124, c6p], bf16, tag="g2m")
            g2sb = gpool.tile([124, c6p], bf16, tag="g2sb")
            out_t = outp.tile([96, c6p], f32, tag="out")
            nc.gpsimd.memset(g2s[:, :], 0.0)
            nc.gpsimd.memset(g2m[:, :], 0.0)

            lvl2_next = {k: 0 for k in classes}  # next slot to reduce
            final_done = set()
            chunk_cls = {
                cc: [k for k in classes
                     if c_k[k] < cc + BLK and c_k[k] + n48[k] > cc]
                for cc in range(0, c6p, BLK)
            }

            def _emit_lvl2(groups_ready):
                # incremental: reduce only the slots whose level-1 groups
                # completed, so each piece stays small and never head-of-line
                # blocks the DVE queue
                for k in classes:
                    nk = n48[k]
                    a = a_k[k]
                    done = lvl2_next[k]
                    if done >= nk:
                        continue
                    ready = min(nk, max(0, (groups_ready - a) // k))
                    if ready <= done:
                        continue
                    c0 = c_k[k]
                    gv_s = g1s[0:124, a + done * k : a + ready * k].rearrange(
                        "p (n k) -> p n k", k=k)
                    gv_m = g1m[0:124, a + done * k : a + ready * k].rearrange(
                        "p (n k) -> p n k", k=k)
                    nc.vector.tensor_reduce(
                        out=g2s[0:124, c0 + done : c0 + ready], in_=gv_s,
                        axis=AX, op=OP.add)
                    nc.vector.tensor_reduce(
                        out=g2m[0:124, c0 + done : c0 + ready], in_=gv_m,
                        axis=AX, op=OP.max)
                    lvl2_next[k] = ready

            def _maybe_final():
                # emit mean-scale + final GEMM + output DMA for any 512-col
                # chunk whose classes have all been level-2 reduced
                for cc in range(0, c6p, BLK):
                    if cc in final_done:
                        continue
                    if not all(lvl2_next[k] >= n48[k] for k in chunk_cls[cc]):
                        continue
                    final_done.add(cc)
                    nc.vector.tensor_mul(out=g2sb[:, cc : cc + BLK],
                                         in0=g2s[:, cc : cc + BLK],
                                         in1=recip_t[:, cc : cc + BLK])
                    po = pbp.tile([128, BLK], f32, tag="pb")
                    nc.tensor.matmul(
                        po[0:96, :], lhsT=woutm_t[:, :],
                        rhs=g2sb[0:124, cc : cc + BLK],
                        start=True, stop=False,
                    )
                    nc.tensor.matmul(
                        po[0:96, :], lhsT=woutx_t[:, :],
                        rhs=g2m[0:124, cc : cc + BLK],
                        start=False, stop=True,
                    )
                    nc.scalar.activation(out_t[0:96, cc : cc + BLK],
                                         po[0:96, :], RELU, bias=bo6_t[:, :])
                    nc.sync.dma_start(out=out_d[:, cc : cc + BLK],
                                      in_=out_t[0:96, cc : cc + BLK])

            for di, (o3, w3) in enumerate(chunk_list):
                o6, w6 = o3 // 2, w3 // 2
                ds3_t = ds3_ts[di % 3]
                xf6_t = xf6_ts[di % 3]
                for bl in range(w6 // BLK):
                    b = o6 // BLK + bl  # global pb-block index
                    # --- stage 1: 3-lane-packed MLPs ---
                    pa = pap.tile([128, 1024], f32, tag="pa")
                    for t in range(2):
                        nc.tensor.matmul(
                            pa[0:120, 512 * t : 512 * t + 512],
                            lhsT=wa3_t[:, :],
                            rhs=ds3_t[:, 1024 * bl + 512 * t
                                      : 1024 * bl + 512 * t + 512],
                            start=True, stop=True,
                        )
                    hx = hx_ts[b % 3]
                    nc.scalar.activation(hx[0:120, :], pa[0:120, :], RELU,
                                         bias=ba3_t[:, :])
                    # --- stage 2: y pre-activation in pb ---
                    pb = pbp.tile([128, BLK], f32, tag="pb")
                    nc.tensor.matmul(
                        pb[0:124, :], lhsT=wxw_t[:, :],
                        rhs=xf6_t[:, BLK * bl : BLK * bl + BLK],
                        start=True, stop=False,
                    )
                    nc.tensor.matmul(
                        pb[0:60, :], lhsT=wh_t[:, 0:60],
                        rhs=hx[:, 0:512],
                        start=False, stop=True, skip_group_check=True,
                    )
                    nc.tensor.matmul(
                        pb[64:124, :], lhsT=wh_t[:, 0:60],
                        rhs=hx[:, 512:1024],
                        start=False, stop=True, skip_group_check=True,
                    )
                    # --- reluB into the quad y tile ---
                    qoff = b % 4
                    if qoff == 0:
                        q0 = b
                        yq = yp.tile([124, 4 * BLK], bf16, tag="yq")
                    ysl = yq[:, BLK * qoff : BLK * qoff + BLK]
                    if b % 8 in RELUB_ACT:
                        nc.scalar.activation(ysl, pb[0:124, :], RELU,
                                             bias=bb6_t[:, :])
                    else:
                        nc.vector.tensor_scalar(
                            out=ysl, in0=pb[0:124, :],
                            scalar1=bb6_t[:, :], scalar2=0.0,
                            op0=OP.add, op1=OP.max)
                    # --- lvl-1 tree, batched over the quad ---
                    if qoff == 3 or b == nblk - 1:
                        nq = b - q0 + 1
                        yv = yq[:, 0 : BLK * nq].rearrange(
                            "p (g k) -> p g k", k=8)
                        t1s = t1p.tile([124, 1024], bf16, tag="t1s")
                        t1m = m1p.tile([124, 1024], bf16, tag="t1m")
                        for t1_, g1_, op_ in ((t1s, g1s, OP.add),
                                              (t1m, g1m, OP.max)):
                            t1v = t1_[:, 0 : 256 * nq].rearrange(
                                "p (g k) -> p g k", k=4)
                            nc.vector.tensor_tensor(
                                out=t1v, in0=yv[:, :, 0:4],
                                in1=yv[:, :, 4:8], op=op_)
                            t2 = t2p.tile([124, 512], bf16, tag="t2")
                            t2v = t2[:, 0 : 128 * nq].rearrange(
                                "p (g k) -> p g k", k=2)
                            nc.vector.tensor_tensor(
                                out=t2v, in0=t1v[:, :, 0:2],
                                in1=t1v[:, :, 2:4], op=op_)
                            nc.vector.tensor_tensor(
                                out=g1_[0:124, 64 * q0 : 64 * (q0 + nq)],
                                in0=t2v[:, :, 0], in1=t2v[:, :, 1], op=op_)
                        if (q0 // 4) % 2 == 1 or b == nblk - 1:
                            _emit_lvl2(64 * (q0 + nq))
                            _maybe_final()

                # prefetch: chunk di+3 reuses this chunk's tiles; emitting
                # the trigger after this chunk's readers gives it the right
                # WAR dependency while still running ~2 chunks ahead.
                if di + 3 < len(chunk_list):
                    trigger_chunk(di + 3)

            _emit_lvl2(G1 * 2)
            _maybe_final()
            assert len(final_done) == c6p // BLK

    nc.compile()
    return nc


# ----------------------------------------------------------------------------
# Entry point
# ----------------------------------------------------------------------------

def _gather_output(core_data, outs):
    OUT = np.zeros((C, D_OUT), dtype=np.float32)
    for ci in range(N_CORES):
        _, _, _, slot_comm = core_data[ci]
        oimg = np.asarray(outs[ci], dtype=np.float32)
        for lj in range(N_LANES):
            comms = slot_comm[lj]
            real = comms >= 0
            OUT[comms[real]] = oimg[16 * lj : 16 * lj + 16, : len(real)][:, real].T
    return OUT


def kernel(x, dataset_x, community, multi_community_nodes, multi_community_index,
           W_demo, b_demo, W_purch, b_purch, W_feat, b_feat, W_out, b_out,
           _run_device=None):
    x = np.asarray(x, dtype=np.float32)
    dataset_x = np.asarray(dataset_x, dtype=np.float32)
    community = np.asarray(community)
    multi_community_nodes = np.asarray(multi_community_nodes)
    multi_community_index = np.asarray(multi_community_index)
    params = tuple(
        np.asarray(p, dtype=np.float32)
        for p in (W_demo, b_demo, W_purch, b_purch, W_feat, b_feat, W_out, b_out)
    )

    core_data, layout = _plan(community, multi_community_index,
                              multi_community_nodes)
    shared = _build_shared_inputs(params)
    in_maps = []
    for ci in range(N_CORES):
        m = _build_core_inputs(core_data[ci], layout, x, dataset_x)
        m.update(shared)
        in_maps.append(m)

    if _run_device is None:
        from concourse.bass_utils import run_bass_kernel_spmd

        nc = _build_nc(layout)
        res = run_bass_kernel_spmd(nc, in_maps, list(range(N_CORES)))
        outs = [res.results[i]["out"] for i in range(N_CORES)]
    else:
        outs = _run_device(layout, in_maps)

    return _gather_output(core_data, outs)


# revision 56
# speedup vs baseline: 1.1532x; 1.1532x over previous
"""DeepWalk community-pooling kernel for 8 trn2 NeuronCores (v2).

Pipeline (per core, SPMD identical program, per-core data):
  host: sort extended rows (N + multi duplicates) by community, pad each
        community to a multiple of 8 rows, deal communities per size-class
        round-robin onto 48 (core, lane) slots (6 lanes/core) so every
        slot has an identical class profile.
  device, per 512-column "pb block" (512 stream indices x 6 lanes = 3072
  rows):
    mmA  : ds3^T 3-lane-packed [60,1024] x wa3 -> pa [120,1024] psum
    hx   : ACT relu+bias -> hx3 bf16 [120,1024]
    mm_h : wh^T x hx3 chunks -> pb[0:60] / pb[64:124] (accumulate)
    mm_xw: wxw^T x xf6 [126,512] 6-lane-packed -> pb (x-contribution +
           pad-flag), one matmul at 6-row/col density
    reluB: (pb + b_feat) relu -> y bf16 [124,512]  (ACT or DVE, balanced)
    sum  : DVE TT-tree radix-8 (2x bf16 mode) -> g1s
    max  : GPSIMD TT-tree radix-8 -> g1m
    lvl2 : per size-class tensor_reduce over k groups -> g2s (f32), g2m
  tail:  mean = g2s * recip (host-provided reciprocals), final GEMM
         relu(W_out^T [mean; max] + b_out) -> out [96, c6p]
  host: gather per-lane outputs back to the global community order.
"""

import sys

import numpy as np

sys.path.insert(0, "/opt/trn_rl_repo")

import ml_dtypes  # noqa: E402

BF16 = ml_dtypes.bfloat16
FP8 = ml_dtypes.float8_e4m3fn

N = 2_000_000
M = 500_000
C = 50_000
D_OUT = 16
N_CORES = 8
N_LANES = 6  # per core
SLOTS = N_CORES * N_LANES
BLK = 512  # pb columns per block
FLAG_PAD = -32768.0
W3_DMA = 8192  # ds3 cols per input DMA tile (= 4096 stream idx)
LANE_OFF = [0, 20, 40, 64, 84, 104]  # partition offset of each lane block
RELUB_ACT_MOD = 3  # pair p with p % MOD == 1 does reluB on ACT, rest DVE
N_WARMUP = 17  # back-to-back warm-up matmuls to flip the PE HAM to 2.4 GHz


# ----------------------------------------------------------------------------
# Host-side planning
# ----------------------------------------------------------------------------

def _plan(community, multi_community_index, multi_community_nodes):
    """Sort/pad/shard rows. Returns per-core row sources + static layout."""
    seg = np.concatenate([community, multi_community_index]).astype(np.int64)
    src = np.concatenate(
        [np.arange(N, dtype=np.int64), multi_community_nodes.astype(np.int64)]
    )

    counts = np.bincount(seg, minlength=C)
    kcls = np.maximum((counts + 7) // 8, 1).astype(np.int64)  # class = #groups
    assert kcls.max() <= 64, f"community too large: {counts.max()} rows"

    order = np.argsort(seg, kind="stable")
    src_sorted = src[order]
    starts = np.zeros(C + 1, dtype=np.int64)
    np.cumsum(counts, out=starts[1:])

    # communities per class, dealt round-robin to 48 (core,lane) slots.
    # Classes are laid out largest-first so the level-2 reductions of the
    # big classes complete early and the final GEMM pipelines with the
    # main loop.
    classes = np.unique(kcls)[::-1]
    slot_comms = [[[] for _ in range(N_LANES)] for _ in range(N_CORES)]
    n48 = {}  # class k -> communities per slot
    for k in classes:
        comms = np.nonzero(kcls == k)[0]
        n48[int(k)] = (len(comms) + SLOTS - 1) // SLOTS
        for i, g in enumerate(comms):
            s = i % SLOTS
            slot_comms[s // N_LANES][s % N_LANES].append(int(g))
    classes = [int(k) for k in classes]

    # per-lane group/community layout (identical across all cores/lanes)
    lane_groups = sum(n48[k] * k for k in classes)
    c6 = sum(n48[k] for k in classes)  # community slots per lane
    c6p = ((c6 + BLK - 1) // BLK) * BLK
    lane_rows = lane_groups * 8
    lane_len = ((lane_rows + BLK - 1) // BLK) * BLK

    # class offsets (group units and community-slot units)
    a_k, c_k, ga, ca = {}, {}, 0, 0
    for k in classes:
        a_k[k] = ga
        c_k[k] = ca
        ga += n48[k] * k
        ca += n48[k]

    # per (core,lane): row source indices (-1 = padding), per-slot counts
    core_data = []
    for ci in range(N_CORES):
        lane_src = np.full((N_LANES, lane_len), -1, dtype=np.int64)
        lane_flag = np.full((N_LANES, lane_len), FLAG_PAD, dtype=np.float32)
        slot_count = np.zeros((N_LANES, c6p), dtype=np.int64)
        slot_comm = np.full((N_LANES, c6p), -1, dtype=np.int64)
        for lj in range(N_LANES):
            comms = slot_comms[ci][lj]
            by_k = {k: [] for k in classes}
            for g in comms:
                by_k[int(kcls[g])].append(g)
            pos = 0
            for k in classes:
                lst = by_k[k]
                for i in range(n48[k]):
                    slot = c_k[k] + i
                    if i < len(lst):
                        g = lst[i]
                        cnt = int(counts[g])
                        s0 = starts[g]
                        lane_src[lj, pos : pos + cnt] = src_sorted[s0 : s0 + cnt]
                        lane_flag[lj, pos : pos + cnt] = 0.0
                        slot_count[lj, slot] = cnt
                        slot_comm[lj, slot] = g
                    pos += 8 * k
            assert pos == lane_rows
        core_data.append((lane_src, lane_flag, slot_count, slot_comm))

    layout = dict(
        classes=classes, n48=n48, a_k=a_k, c_k=c_k,
        c6=c6, c6p=c6p, lane_len=lane_len, lane_groups=lane_groups,
    )
    return core_data, layout


def _build_core_inputs(core_dat, layout, x, dataset_x):
    """Build the DRAM images for one core."""
    lane_src, lane_flag, slot_count, _ = core_dat
    lane_len = layout["lane_len"]
    c6p = layout["c6p"]
    nblk = lane_len // BLK
    F3 = 2 * lane_len
    F6 = lane_len

    idx = np.maximum(lane_src, 0)

    # ds3 [60, F3]: col 1024b+512t+j holds lanes {3t,3t+1,3t+2} at stream
    # index 512b+j; lane 3t+m occupies partitions 20m..20m+20. fp8: the
    # demo/purch MLP path tolerates e4m3 (verified ~0.004 end-to-end).
    arr = dataset_x[idx].astype(FP8)               # [6, lane_len, 20]
    arrv = arr.reshape(2, 3, nblk, BLK, 20)        # [t, m, b, j, f]
    ds3 = np.zeros((64, F3), dtype=FP8)
    ds3[0:60] = arrv.transpose(1, 4, 2, 0, 3).reshape(60, F3)

    # xf6 [126, F6]: col i holds all 6 lanes at stream index i;
    # lane l occupies partitions 21l..21l+20 (+ flag channel at 21l+20).
    xv = x[idx].astype(BF16)                       # [6, lane_len, 20]
    xf6 = np.zeros((128, F6), dtype=BF16)
    for l in range(N_LANES):
        xf6[21 * l : 21 * l + 20] = xv[l].T
        xf6[21 * l + 20] = lane_flag[l].astype(BF16)

    recip = np.ones((124, c6p), dtype=np.float32)
    for l in range(N_LANES):
        r = 1.0 / np.maximum(slot_count[l], 1).astype(np.float32)
        off = LANE_OFF[l]
        recip[off : off + 20, :] = r[None, :]

    return dict(ds3=ds3, xf6=xf6, recip=recip)


def _build_shared_inputs(params):
    (W_demo, b_demo, W_purch, b_purch, W_feat, b_feat, W_out, b_out) = params

    # mmA stationary [128, 120]: 3 lanes; lane t ds feats at partitions
    # 20t..20t+20 -> h (demo|purch) at out cols 40t..40t+40. All matmul
    # contracts are zero-padded to the full 128 rows: the PE HAM activity
    # monitor only un-throttles the clock gate (1.2 -> 2.4 GHz) for
    # full-height operands.
    wa3 = np.zeros((128, 120), dtype=FP8)
    for t in range(3):
        wa3[20 * t : 20 * t + 8, 40 * t : 40 * t + 20] = W_demo
        wa3[20 * t + 8 : 20 * t + 20, 40 * t + 20 : 40 * t + 40] = W_purch

    ba3 = np.zeros((120, 1), dtype=np.float32)
    for t in range(3):
        ba3[40 * t : 40 * t + 20, 0] = b_demo
        ba3[40 * t + 20 : 40 * t + 40, 0] = b_purch

    # mm_h stationary [128, 60]: lane t h-feats at 40t..40t+40 -> y cols
    # 20t..20t+20 (chunk A lanes 0-2 at pb[0:60], chunk B lanes 3-5 at
    # pb[64:124])
    wh = np.zeros((128, 60), dtype=BF16)
    for t in range(3):
        wh[40 * t : 40 * t + 40, 20 * t : 20 * t + 20] = W_feat[0:40]

    # mm_xw stationary [128, 124]: 6-lane-packed x -> x-part of y, plus the
    # pad flag channel -> -32768 on that lane's 20 y cols
    wxw = np.zeros((128, 124), dtype=BF16)
    for l in range(N_LANES):
        off = LANE_OFF[l]
        wxw[21 * l : 21 * l + 20, off : off + 20] = W_feat[40:60]
        wxw[21 * l + 20, off : off + 20] = 1.0

    bb6 = np.zeros((124, 1), dtype=np.float32)
    for l in range(N_LANES):
        off = LANE_OFF[l]
        bb6[off : off + 20, 0] = b_feat

    # final GEMM stationaries [124, 96]: lane l mean/max rows -> out cols
    # 16l..16l+16
    woutm = np.zeros((124, 96), dtype=BF16)
    woutx = np.zeros((124, 96), dtype=BF16)
    for l in range(N_LANES):
        off = LANE_OFF[l]
        woutm[off : off + 20, 16 * l : 16 * l + 16] = W_out[0:20]
        woutx[off : off + 20, 16 * l : 16 * l + 16] = W_out[20:40]

    bo6 = np.zeros((96, 1), dtype=np.float32)
    for l in range(N_LANES):
        bo6[16 * l : 16 * l + 16, 0] = b_out

    return dict(wa3=wa3, ba3=ba3, wh=wh, wxw=wxw, bb6=bb6,
                woutm=woutm, woutx=woutx, bo6=bo6)


# ----------------------------------------------------------------------------
# Device kernel
# ----------------------------------------------------------------------------

def _build_nc(layout):
    import concourse.bacc as bacc
    import concourse.mybir as mybir
    from concourse import tile

    f32 = mybir.dt.float32
    bf16 = mybir.dt.bfloat16
    f8 = mybir.dt.float8e4

    lane_len = layout["lane_len"]
    c6p = layout["c6p"]
    nblk = lane_len // BLK
    F3 = 2 * lane_len
    F6 = lane_len
    G1 = nblk * 64  # lvl-1 group columns (64 per block)
    classes = layout["classes"]
    n48 = layout["n48"]
    a_k = layout["a_k"]
    c_k = layout["c_k"]

    nc = bacc.Bacc("TRN2", target_bir_lowering=False, debug=False)

    dt_map = dict(ds3=f8, xf6=bf16, recip=f32, wa3=f8, wh=bf16, wxw=bf16,
                  woutm=bf16, woutx=bf16, ba3=f32, bb6=f32, bo6=f32)
    shapes = dict(ds3=[64, F3], xf6=[128, F6], recip=[124, c6p],
                  wa3=[128, 120], wh=[128, 60], wxw=[128, 124],
                  woutm=[124, 96], woutx=[124, 96],
                  ba3=[120, 1], bb6=[124, 1], bo6=[96, 1])
    dram = {
        name: nc.declare_dram_parameter(name, shapes[name], dt_map[name],
                                        isOutput=False)
        for name in shapes
    }
    out_d = nc.declare_dram_parameter("out", [96, c6p], f32, isOutput=True)

    AX = mybir.AxisListType.X
    OP = mybir.AluOpType
    RELU = mybir.ActivationFunctionType.Relu

    with tile.TileContext(nc) as tc:
        with (
            tc.tile_pool(name="wpool", bufs=1) as wpool,
            tc.tile_pool(name="g", bufs=1) as gpool,
            tc.tile_pool(name="ds3p", bufs=2) as ds3p,
            tc.tile_pool(name="xf6p", bufs=2) as xf6p,
            tc.tile_pool(name="hxp", bufs=3) as hxp,
            tc.tile_pool(name="yp", bufs=3) as yp,
            tc.tile_pool(name="t1p", bufs=2) as t1p,
            tc.tile_pool(name="t2p", bufs=4) as t2p,
            tc.tile_pool(name="m1p", bufs=2) as m1p,
            tc.tile_pool(name="pa", bufs=2, space="PSUM") as pap,
            tc.tile_pool(name="pb", bufs=2, space="PSUM") as pbp,
            tc.tile_pool(name="outp", bufs=1) as outp,
        ):
            wa3_t = wpool.tile([128, 120], f8, tag="wa3")
            wh_t = wpool.tile([128, 60], bf16, tag="wh")
            wxw_t = wpool.tile([128, 124], bf16, tag="wxw")
            woutm_t = wpool.tile([124, 96], bf16, tag="woutm")
            woutx_t = wpool.tile([124, 96], bf16, tag="woutx")
            ba3_t = wpool.tile([120, 1], f32, tag="ba3")
            bb6_t = wpool.tile([124, 1], f32, tag="bb6")
            bo6_t = wpool.tile([96, 1], f32, tag="bo6")
            recip_t = wpool.tile([124, c6p], f32, tag="recip")

            # fixed hand-rotated input/hx tiles, zero-padded to 128 rows so
            # every matmul streams a full-height rhs (HAM activity)
            ds3_ts = [wpool.tile([128, W3_DMA], f8, tag=f"ds3{i}",
                                 name=f"ds3{i}") for i in range(3)]
            xf6_ts = [wpool.tile([128, W3_DMA // 2], bf16, tag=f"xf6{i}",
                                 name=f"xf6{i}") for i in range(3)]
            hx_ts = [wpool.tile([128, 1024], bf16, tag=f"hx{i}",
                                name=f"hx{i}") for i in range(3)]
            wtmp = wpool.tile([128, 512], bf16, tag="wtmp")
            nc.gpsimd.memset(wtmp[:, :], 0.0)
            # the first 4096 cols gate chunk 0's matmuls; the rest of tile 0
            # is only read from chunk 3 on, so zero it off the critical path
            nc.gpsimd.memset(ds3_ts[0][64:128, 0:4096], 0.0)
            nc.gpsimd.memset(ds3_ts[0][64:128, 4096:W3_DMA], 0.0)
            nc.gpsimd.memset(ds3_ts[1][64:128, :], 0.0)
            nc.vector.memset(ds3_ts[2][64:128, :], 0.0)
            nc.scalar.memzero(hx_ts[0][96:128, :])
            nc.scalar.memzero(hx_ts[1][96:128, :])
            nc.scalar.memzero(hx_ts[2][96:128, :])

            # input chunk schedule: two small leading chunks cut the
            # time-to-first-block; input triggers precede the bulky weight
            # transfers on the sync queue.
            chunk_list = []
            o3 = 0
            while o3 < F3:
                w3 = min(4096 if o3 < 8192 else W3_DMA, F3 - o3)
                chunk_list.append((o3, w3))
                o3 += w3

            def trigger_chunk(di):
                o3, w3 = chunk_list[di]
                ds3_t = ds3_ts[di % 3]
                xf6_t = xf6_ts[di % 3]
                nc.sync.dma_start(out=ds3_t[0:64, :w3],
                                  in_=dram["ds3"][:, o3 : o3 + w3])
                nc.sync.dma_start(out=xf6_t[0:128, : w3 // 2],
                                  in_=dram["xf6"][:, o3 // 2 : (o3 + w3) // 2])

            trigger_chunk(0)
            for name, t in [("wa3", wa3_t), ("ba3", ba3_t)]:
                nc.sync.dma_start(out=t[:], in_=dram[name][:])

            # PE HAM warm-up, gated on the first input chunk (via the copy
            # below): back-to-back full-128x128 matmuls give the activity
            # monitor a fully-busy 4096-cycle window, flipping the PE clock
            # gate from its default 1.2 GHz to 2.4 GHz; the main loop's own
            # dense stream keeps it warm from there.
            nc.vector.tensor_copy(out=wtmp[0:1, 0:1], in_=ds3_ts[0][0:1, 0:1])
            pw0 = pbp.tile([128, BLK], f32, tag="pb")
            for _ in range(N_WARMUP):
                nc.tensor.matmul(pw0[:, :], lhsT=wtmp[:, 0:128],
                                 rhs=wtmp[:, :], start=True, stop=True)

            for name, t in [("wh", wh_t), ("wxw", wxw_t), ("bb6", bb6_t)]:
                nc.sync.dma_start(out=t[:], in_=dram[name][:])
            trigger_chunk(1)
            for name, t in [("woutm", woutm_t), ("woutx", woutx_t),
                            ("bo6", bo6_t)]:
                nc.sync.dma_start(out=t[:], in_=dram[name][:])
            trigger_chunk(2)
            nc.sync.dma_start(out=recip_t[:], in_=dram["recip"][:])

            g1s = gpool.tile([124, G1], bf16, tag="g1s")
            g1m = gpool.tile([124, G1], bf16, tag="g1m")
            g2s = gpool.tile([124, c6p], f32, tag="g2s")
            g2m = gpool.tile([124, c6p], bf16, tag="g2m")
            g2sb = gpool.tile([124, c6p], bf16, tag="g2sb")
            out_t = outp.tile([96, c6p], f32, tag="out")
            nc.gpsimd.memset(g2s[:, :], 0.0)
            nc.gpsimd.memset(g2m[:, :], 0.0)

            lvl2_next = {k: 0 for k in classes}  # next slot to reduce
            final_done = set()
            chunk_cls = {
                cc: [k for k in classes
                     if c_k[k] < cc + BLK and c_k[k] + n48[k] > cc]
                for cc in range(0, c6p, BLK)
            }

            def _emit_lvl2(groups_ready):
                # incremental: reduce only the slots whose level-1 groups
                # completed, so each piece stays small and never head-of-line
                # blocks the DVE queue
                for k in classes:
                    nk = n48[k]
                    a = a_k[k]
                    done = lvl2_next[k]
                    if done >= nk:
                        continue
                    ready = min(nk, max(0, (groups_ready - a) // k))
                    if ready <= done:
                        continue
                    c0 = c_k[k]
                    gv_s = g1s[0:124, a + done * k : a + ready * k].rearrange(
                        "p (n k) -> p n k", k=k)
                    gv_m = g1m[0:124, a + done * k : a + ready * k].rearrange(
                        "p (n k) -> p n k", k=k)
                    nc.vector.tensor_reduce(
                        out=g2s[0:124, c0 + done : c0 + ready], in_=gv_s,
                        axis=AX, op=OP.add)
                    nc.vector.tensor_reduce(
                        out=g2m[0:124, c0 + done : c0 + ready], in_=gv_m,
                        axis=AX, op=OP.max)
                    lvl2_next[k] = ready

            def _maybe_final():
                # emit mean-scale + final GEMM + output DMA for any 512-col
                # chunk whose classes have all been level-2 reduced
                for cc in range(0, c6p, BLK):
                    if cc in final_done:
                        continue
                    if not all(lvl2_next[k] >= n48[k] for k in chunk_cls[cc]):
                        continue
                    final_done.add(cc)
                    nc.vector.tensor_mul(out=g2sb[:, cc : cc + BLK],
                                         in0=g2s[:, cc : cc + BLK],
                                         in1=recip_t[:, cc : cc + BLK])
                    po = pbp.tile([128, BLK], f32, tag="pb")
                    nc.tensor.matmul(
                        po[0:96, :], lhsT=woutm_t[:, :],
                        rhs=g2sb[0:124, cc : cc + BLK],
                        start=True, stop=False,
                    )
                    nc.tensor.matmul(
                        po[0:96, :], lhsT=woutx_t[:, :],
                        rhs=g2m[0:124, cc : cc + BLK],
                        start=False, stop=True,
                    )
                    nc.scalar.activation(out_t[0:96, cc : cc + BLK],
                                         po[0:96, :], RELU, bias=bo6_t[:, :])
                    nc.sync.dma_start(out=out_d[:, cc : cc + BLK],
                                      in_=out_t[0:96, cc : cc + BLK])

            for di, (o3, w3) in enumerate(chunk_list):
                o6, w6 = o3 // 2, w3 // 2
                ds3_t = ds3_ts[di % 3]
                xf6_t = xf6_ts[di % 3]
                for bl in range(w6 // BLK):
                    b = o6 // BLK + bl  # global pb-block index
                    # --- stage 1: 3-lane-packed MLPs ---
                    pa = pap.tile([128, 1024], f32, tag="pa")
                    for t in range(2):
                        nc.tensor.matmul(
                            pa[0:120, 512 * t : 512 * t + 512],
                            lhsT=wa3_t[:, :],
                            rhs=ds3_t[:, 1024 * bl + 512 * t
                                      : 1024 * bl + 512 * t + 512],
                            start=True, stop=True,
                        )
                    hx = hx_ts[b % 3]
                    nc.scalar.activation(hx[0:120, :], pa[0:120, :], RELU,
                                         bias=ba3_t[:, :])
                    # --- stage 2: y pre-activation, two blocks per pb pair ---
                    poff = b % 2
                    if poff == 0:
                        pb = pbp.tile([128, 2 * BLK], f32, tag="pb")
                    ph = pb[:, BLK * poff : BLK * poff + BLK]
                    nc.tensor.matmul(
                        ph[0:124, :], lhsT=wxw_t[:, :],
                        rhs=xf6_t[:, BLK * bl : BLK * bl + BLK],
                        start=True, stop=False,
                    )
                    nc.tensor.matmul(
                        ph[0:60, :], lhsT=wh_t[:, 0:60],
                        rhs=hx[:, 0:512],
                        start=False, stop=True, skip_group_check=True,
                    )
                    nc.tensor.matmul(
                        ph[64:124, :], lhsT=wh_t[:, 0:60],
                        rhs=hx[:, 512:1024],
                        start=False, stop=True, skip_group_check=True,
                    )
                    # --- reluB over the pair, into the octet y tile ---
                    ooff = b % 8
                    if ooff == 0:
                        q0 = b
                        yq = yp.tile([124, 8 * BLK], bf16, tag="yq")
                    if poff == 1 or b == nblk - 1:
                        w_pair = BLK * (poff + 1)
                        ysl = yq[:, BLK * (ooff - poff)
                                 : BLK * (ooff - poff) + w_pair]
                        if (b // 2) % RELUB_ACT_MOD == 1:
                            nc.scalar.activation(ysl, pb[0:124, :w_pair],
                                                 RELU, bias=bb6_t[:, :])
                        else:
                            nc.vector.tensor_scalar(
                                out=ysl, in0=pb[0:124, :w_pair],
                                scalar1=bb6_t[:, :], scalar2=0.0,
                                op0=OP.add, op1=OP.max)
                    # --- lvl-1 tree, batched over the octet ---
                    if ooff == 7 or b == nblk - 1:
                        nq = b - q0 + 1
                        yv = yq[:, 0 : BLK * nq].rearrange(
                            "p (g k) -> p g k", k=8)
                        t1s = t1p.tile([124, 2048], bf16, tag="t1s")
                        t1m = m1p.tile([124, 2048], bf16, tag="t1m")
                        for t1_, g1_, op_ in ((t1s, g1s, OP.add),
                                              (t1m, g1m, OP.max)):
                            t1v = t1_[:, 0 : 256 * nq].rearrange(
                                "p (g k) -> p g k", k=4)
                            nc.vector.tensor_tensor(
                                out=t1v, in0=yv[:, :, 0:4],
                                in1=yv[:, :, 4:8], op=op_)
                            t2 = t2p.tile([124, 1024], bf16, tag="t2")
                            t2v = t2[:, 0 : 128 * nq].rearrange(
                                "p (g k) -> p g k", k=2)
                            nc.vector.tensor_tensor(
                                out=t2v, in0=t1v[:, :, 0:2],
                                in1=t1v[:, :, 2:4], op=op_)
                            nc.vector.tensor_tensor(
                                out=g1_[0:124, 64 * q0 : 64 * (q0 + nq)],
                                in0=t2v[:, :, 0], in1=t2v[:, :, 1], op=op_)
                        _emit_lvl2(64 * (q0 + nq))
                        _maybe_final()

                # prefetch: chunk di+3 reuses this chunk's tiles; emitting
                # the trigger after this chunk's readers gives it the right
                # WAR dependency while still running ~2 chunks ahead.
                if di + 3 < len(chunk_list):
                    trigger_chunk(di + 3)

            _emit_lvl2(G1 * 2)
            _maybe_final()
            assert len(final_done) == c6p // BLK

    nc.compile()
    return nc


# ----------------------------------------------------------------------------
# Entry point
# ----------------------------------------------------------------------------

def _gather_output(core_data, outs):
    OUT = np.zeros((C, D_OUT), dtype=np.float32)
    for ci in range(N_CORES):
        _, _, _, slot_comm = core_data[ci]
        oimg = np.asarray(outs[ci], dtype=np.float32)
        for lj in range(N_LANES):
            comms = slot_comm[lj]
            real = comms >= 0
            OUT[comms[real]] = oimg[16 * lj : 16 * lj + 16, : len(real)][:, real].T
    return OUT


def kernel(x, dataset_x, community, multi_community_nodes, multi_community_index,
           W_demo, b_demo, W_purch, b_purch, W_feat, b_feat, W_out, b_out,
           _run_device=None):
    x = np.asarray(x, dtype=np.float32)
    dataset_x = np.asarray(dataset_x, dtype=np.float32)
    community = np.asarray(community)
    multi_community_nodes = np.asarray(multi_community_nodes)
    multi_community_index = np.asarray(multi_community_index)
    params = tuple(
        np.asarray(p, dtype=np.float32)
        for p in (W_demo, b_demo, W_purch, b_purch, W_feat, b_feat, W_out, b_out)
    )

    core_data, layout = _plan(community, multi_community_index,
                              multi_community_nodes)
    shared = _build_shared_inputs(params)
    in_maps = []
    for ci in range(N_CORES):
        m = _build_core_inputs(core_data[ci], layout, x, dataset_x)
        m.update(shared)
        in_maps.append(m)

    if _run_device is None:
        from concourse.bass_utils import run_bass_kernel_spmd

        nc = _build_nc(layout)
        res = run_bass_kernel_spmd(nc, in_maps, list(range(N_CORES)))
        outs = [res.results[i]["out"] for i in range(N_CORES)]
    else:
        outs = _run_device(layout, in_maps)

    return _gather_output(core_data, outs)


# revision 58
# speedup vs baseline: 1.1778x; 1.0213x over previous
"""DeepWalk community-pooling kernel for 8 trn2 NeuronCores (v2).

Pipeline (per core, SPMD identical program, per-core data):
  host: sort extended rows (N + multi duplicates) by community, pad each
        community to a multiple of 8 rows, deal communities per size-class
        round-robin onto 48 (core, lane) slots (6 lanes/core) so every
        slot has an identical class profile.
  device, per 512-column "pb block" (512 stream indices x 6 lanes = 3072
  rows):
    mmA  : ds3^T 3-lane-packed [60,1024] x wa3 -> pa [120,1024] psum
    hx   : ACT relu+bias -> hx3 bf16 [120,1024]
    mm_h : wh^T x hx3 chunks -> pb[0:60] / pb[64:124] (accumulate)
    mm_xw: wxw^T x xf6 [126,512] 6-lane-packed -> pb (x-contribution +
           pad-flag), one matmul at 6-row/col density
    reluB: (pb + b_feat) relu -> y bf16 [124,512]  (ACT or DVE, balanced)
    sum  : DVE TT-tree radix-8 (2x bf16 mode) -> g1s
    max  : GPSIMD TT-tree radix-8 -> g1m
    lvl2 : per size-class tensor_reduce over k groups -> g2s (f32), g2m
  tail:  mean = g2s * recip (host-provided reciprocals), final GEMM
         relu(W_out^T [mean; max] + b_out) -> out [96, c6p]
  host: gather per-lane outputs back to the global community order.
"""

import sys

import numpy as np

sys.path.insert(0, "/opt/trn_rl_repo")

import ml_dtypes  # noqa: E402

BF16 = ml_dtypes.bfloat16
FP8 = ml_dtypes.float8_e4m3fn

N = 2_000_000
M = 500_000
C = 50_000
D_OUT = 16
N_CORES = 8
N_LANES = 6  # per core
SLOTS = N_CORES * N_LANES
BLK = 512  # pb columns per block
FLAG_PAD = -32768.0
W3_DMA = 8192  # ds3 cols per input DMA tile (= 4096 stream idx)
LANE_OFF = [0, 20, 40, 64, 84, 104]  # partition offset of each lane block
RELUB_ACT_MOD = 4  # pair p with p % MOD == 1 does reluB on ACT, rest DVE
N_WARMUP = 17  # back-to-back warm-up matmuls to flip the PE HAM to 2.4 GHz


# ----------------------------------------------------------------------------
# Host-side planning
# ----------------------------------------------------------------------------

def _plan(community, multi_community_index, multi_community_nodes):
    """Sort/pad/shard rows. Returns per-core row sources + static layout."""
    seg = np.concatenate([community, multi_community_index]).astype(np.int64)
    src = np.concatenate(
        [np.arange(N, dtype=np.int64), multi_community_nodes.astype(np.int64)]
    )

    counts = np.bincount(seg, minlength=C)
    kcls = np.maximum((counts + 7) // 8, 1).astype(np.int64)  # class = #groups
    assert kcls.max() <= 64, f"community too large: {counts.max()} rows"

    order = np.argsort(seg, kind="stable")
    src_sorted = src[order]
    starts = np.zeros(C + 1, dtype=np.int64)
    np.cumsum(counts, out=starts[1:])

    # communities per class, dealt round-robin to 48 (core,lane) slots.
    # Classes are laid out largest-first so the level-2 reductions of the
    # big classes complete early and the final GEMM pipelines with the
    # main loop.
    classes = np.unique(kcls)[::-1]
    slot_comms = [[[] for _ in range(N_LANES)] for _ in range(N_CORES)]
    n48 = {}  # class k -> communities per slot
    for k in classes:
        comms = np.nonzero(kcls == k)[0]
        n48[int(k)] = (len(comms) + SLOTS - 1) // SLOTS
        for i, g in enumerate(comms):
            s = i % SLOTS
            slot_comms[s // N_LANES][s % N_LANES].append(int(g))
    classes = [int(k) for k in classes]

    # per-lane group/community layout (identical across all cores/lanes)
    lane_groups = sum(n48[k] * k for k in classes)
    c6 = sum(n48[k] for k in classes)  # community slots per lane
    c6p = ((c6 + BLK - 1) // BLK) * BLK
    lane_rows = lane_groups * 8
    lane_len = ((lane_rows + BLK - 1) // BLK) * BLK

    # class offsets (group units and community-slot units)
    a_k, c_k, ga, ca = {}, {}, 0, 0
    for k in classes:
        a_k[k] = ga
        c_k[k] = ca
        ga += n48[k] * k
        ca += n48[k]

    # per (core,lane): row source indices (-1 = padding), per-slot counts
    core_data = []
    for ci in range(N_CORES):
        lane_src = np.full((N_LANES, lane_len), -1, dtype=np.int64)
        lane_flag = np.full((N_LANES, lane_len), FLAG_PAD, dtype=np.float32)
        slot_count = np.zeros((N_LANES, c6p), dtype=np.int64)
        slot_comm = np.full((N_LANES, c6p), -1, dtype=np.int64)
        for lj in range(N_LANES):
            comms = slot_comms[ci][lj]
            by_k = {k: [] for k in classes}
            for g in comms:
                by_k[int(kcls[g])].append(g)
            pos = 0
            for k in classes:
                lst = by_k[k]
                for i in range(n48[k]):
                    slot = c_k[k] + i
                    if i < len(lst):
                        g = lst[i]
                        cnt = int(counts[g])
                        s0 = starts[g]
                        lane_src[lj, pos : pos + cnt] = src_sorted[s0 : s0 + cnt]
                        lane_flag[lj, pos : pos + cnt] = 0.0
                        slot_count[lj, slot] = cnt
                        slot_comm[lj, slot] = g
                    pos += 8 * k
            assert pos == lane_rows
        core_data.append((lane_src, lane_flag, slot_count, slot_comm))

    layout = dict(
        classes=classes, n48=n48, a_k=a_k, c_k=c_k,
        c6=c6, c6p=c6p, lane_len=lane_len, lane_groups=lane_groups,
    )
    return core_data, layout


def _build_core_inputs(core_dat, layout, x, dataset_x):
    """Build the DRAM images for one core."""
    lane_src, lane_flag, slot_count, _ = core_dat
    lane_len = layout["lane_len"]
    c6p = layout["c6p"]
    nblk = lane_len // BLK
    F3 = 2 * lane_len
    F6 = lane_len

    idx = np.maximum(lane_src, 0)

    # ds3 [60, F3]: col 1024b+512t+j holds lanes {3t,3t+1,3t+2} at stream
    # index 512b+j; lane 3t+m occupies partitions 20m..20m+20. fp8: the
    # demo/purch MLP path tolerates e4m3 (verified ~0.004 end-to-end).
    arr = dataset_x[idx].astype(FP8)               # [6, lane_len, 20]
    arrv = arr.reshape(2, 3, nblk, BLK, 20)        # [t, m, b, j, f]
    ds3 = np.zeros((64, F3), dtype=FP8)
    ds3[0:60] = arrv.transpose(1, 4, 2, 0, 3).reshape(60, F3)

    # xf6 [126, F6]: col i holds all 6 lanes at stream index i;
    # lane l occupies partitions 21l..21l+20 (+ flag channel at 21l+20).
    xv = x[idx].astype(BF16)                       # [6, lane_len, 20]
    xf6 = np.zeros((128, F6), dtype=BF16)
    for l in range(N_LANES):
        xf6[21 * l : 21 * l + 20] = xv[l].T
        xf6[21 * l + 20] = lane_flag[l].astype(BF16)

    recip = np.ones((124, c6p), dtype=np.float32)
    for l in range(N_LANES):
        r = 1.0 / np.maximum(slot_count[l], 1).astype(np.float32)
        off = LANE_OFF[l]
        recip[off : off + 20, :] = r[None, :]

    return dict(ds3=ds3, xf6=xf6, recip=recip)


def _build_shared_inputs(params):
    (W_demo, b_demo, W_purch, b_purch, W_feat, b_feat, W_out, b_out) = params

    # mmA stationary [128, 120]: 3 lanes; lane t ds feats at partitions
    # 20t..20t+20 -> h (demo|purch) at out cols 40t..40t+40. All matmul
    # contracts are zero-padded to the full 128 rows: the PE HAM activity
    # monitor only un-throttles the clock gate (1.2 -> 2.4 GHz) for
    # full-height operands.
    wa3 = np.zeros((128, 120), dtype=FP8)
    for t in range(3):
        wa3[20 * t : 20 * t + 8, 40 * t : 40 * t + 20] = W_demo
        wa3[20 * t + 8 : 20 * t + 20, 40 * t + 20 : 40 * t + 40] = W_purch

    ba3 = np.zeros((120, 1), dtype=np.float32)
    for t in range(3):
        ba3[40 * t : 40 * t + 20, 0] = b_demo
        ba3[40 * t + 20 : 40 * t + 40, 0] = b_purch

    # mm_h stationary [128, 60]: lane t h-feats at 40t..40t+40 -> y cols
    # 20t..20t+20 (chunk A lanes 0-2 at pb[0:60], chunk B lanes 3-5 at
    # pb[64:124])
    wh = np.zeros((128, 60), dtype=BF16)
    for t in range(3):
        wh[40 * t : 40 * t + 40, 20 * t : 20 * t + 20] = W_feat[0:40]

    # mm_xw stationary [128, 124]: 6-lane-packed x -> x-part of y, plus the
    # pad flag channel -> -32768 on that lane's 20 y cols
    wxw = np.zeros((128, 124), dtype=BF16)
    for l in range(N_LANES):
        off = LANE_OFF[l]
        wxw[21 * l : 21 * l + 20, off : off + 20] = W_feat[40:60]
        wxw[21 * l + 20, off : off + 20] = 1.0

    bb6 = np.zeros((124, 1), dtype=np.float32)
    for l in range(N_LANES):
        off = LANE_OFF[l]
        bb6[off : off + 20, 0] = b_feat

    # final GEMM stationaries [124, 96]: lane l mean/max rows -> out cols
    # 16l..16l+16
    woutm = np.zeros((124, 96), dtype=BF16)
    woutx = np.zeros((124, 96), dtype=BF16)
    for l in range(N_LANES):
        off = LANE_OFF[l]
        woutm[off : off + 20, 16 * l : 16 * l + 16] = W_out[0:20]
        woutx[off : off + 20, 16 * l : 16 * l + 16] = W_out[20:40]

    bo6 = np.zeros((96, 1), dtype=np.float32)
    for l in range(N_LANES):
        bo6[16 * l : 16 * l + 16, 0] = b_out

    return dict(wa3=wa3, ba3=ba3, wh=wh, wxw=wxw, bb6=bb6,
                woutm=woutm, woutx=woutx, bo6=bo6)


# ----------------------------------------------------------------------------
# Device kernel
# ----------------------------------------------------------------------------

def _build_nc(layout):
    import concourse.bacc as bacc
    import concourse.mybir as mybir
    from concourse import tile

    f32 = mybir.dt.float32
    bf16 = mybir.dt.bfloat16
    f8 = mybir.dt.float8e4

    lane_len = layout["lane_len"]
    c6p = layout["c6p"]
    nblk = lane_len // BLK
    F3 = 2 * lane_len
    F6 = lane_len
    G1 = nblk * 64  # lvl-1 group columns (64 per block)
    classes = layout["classes"]
    n48 = layout["n48"]
    a_k = layout["a_k"]
    c_k = layout["c_k"]

    nc = bacc.Bacc("TRN2", target_bir_lowering=False, debug=False)

    dt_map = dict(ds3=f8, xf6=bf16, recip=f32, wa3=f8, wh=bf16, wxw=bf16,
                  woutm=bf16, woutx=bf16, ba3=f32, bb6=f32, bo6=f32)
    shapes = dict(ds3=[64, F3], xf6=[128, F6], recip=[124, c6p],
                  wa3=[128, 120], wh=[128, 60], wxw=[128, 124],
                  woutm=[124, 96], woutx=[124, 96],
                  ba3=[120, 1], bb6=[124, 1], bo6=[96, 1])
    dram = {
        name: nc.declare_dram_parameter(name, shapes[name], dt_map[name],
                                        isOutput=False)
        for name in shapes
    }
    out_d = nc.declare_dram_parameter("out", [96, c6p], f32, isOutput=True)

    AX = mybir.AxisListType.X
    OP = mybir.AluOpType
    RELU = mybir.ActivationFunctionType.Relu

    with tile.TileContext(nc) as tc:
        with (
            tc.tile_pool(name="wpool", bufs=1) as wpool,
            tc.tile_pool(name="g", bufs=1) as gpool,
            tc.tile_pool(name="ds3p", bufs=2) as ds3p,
            tc.tile_pool(name="xf6p", bufs=2) as xf6p,
            tc.tile_pool(name="hxp", bufs=3) as hxp,
            tc.tile_pool(name="yp", bufs=3) as yp,
            tc.tile_pool(name="t1p", bufs=2) as t1p,
            tc.tile_pool(name="t2p", bufs=4) as t2p,
            tc.tile_pool(name="m1p", bufs=2) as m1p,
            tc.tile_pool(name="pa", bufs=2, space="PSUM") as pap,
            tc.tile_pool(name="pb", bufs=2, space="PSUM") as pbp,
            tc.tile_pool(name="outp", bufs=1) as outp,
        ):
            wa3_t = wpool.tile([128, 120], f8, tag="wa3")
            wh_t = wpool.tile([128, 60], bf16, tag="wh")
            wxw_t = wpool.tile([128, 124], bf16, tag="wxw")
            woutm_t = wpool.tile([124, 96], bf16, tag="woutm")
            woutx_t = wpool.tile([124, 96], bf16, tag="woutx")
            ba3_t = wpool.tile([120, 1], f32, tag="ba3")
            bb6_t = wpool.tile([124, 1], f32, tag="bb6")
            bo6_t = wpool.tile([96, 1], f32, tag="bo6")
            recip_t = wpool.tile([124, c6p], f32, tag="recip")

            # fixed hand-rotated input/hx tiles, zero-padded to 128 rows so
            # every matmul streams a full-height rhs (HAM activity)
            ds3_ts = [wpool.tile([128, W3_DMA], f8, tag=f"ds3{i}",
                                 name=f"ds3{i}") for i in range(3)]
            xf6_ts = [wpool.tile([128, W3_DMA // 2], bf16, tag=f"xf6{i}",
                                 name=f"xf6{i}") for i in range(3)]
            hx_ts = [wpool.tile([128, 1024], bf16, tag=f"hx{i}",
                                name=f"hx{i}") for i in range(3)]
            wtmp = wpool.tile([128, 512], bf16, tag="wtmp")
            nc.gpsimd.memset(wtmp[:, :], 0.0)
            # the first 4096 cols gate chunk 0's matmuls; the rest of tile 0
            # is only read from chunk 3 on, so zero it off the critical path
            nc.gpsimd.memset(ds3_ts[0][64:128, 0:4096], 0.0)
            nc.gpsimd.memset(ds3_ts[0][64:128, 4096:W3_DMA], 0.0)
            nc.gpsimd.memset(ds3_ts[1][64:128, :], 0.0)
            nc.vector.memset(ds3_ts[2][64:128, :], 0.0)
            nc.scalar.memzero(hx_ts[0][96:128, :])
            nc.scalar.memzero(hx_ts[1][96:128, :])
            nc.scalar.memzero(hx_ts[2][96:128, :])

            # input chunk schedule: two small leading chunks cut the
            # time-to-first-block; input triggers precede the bulky weight
            # transfers on the sync queue.
            chunk_list = []
            o3 = 0
            while o3 < F3:
                w3 = min(4096 if o3 < 8192 else W3_DMA, F3 - o3)
                chunk_list.append((o3, w3))
                o3 += w3

            def trigger_chunk(di):
                o3, w3 = chunk_list[di]
                ds3_t = ds3_ts[di % 3]
                xf6_t = xf6_ts[di % 3]
                nc.sync.dma_start(out=ds3_t[0:64, :w3],
                                  in_=dram["ds3"][:, o3 : o3 + w3])
                nc.sync.dma_start(out=xf6_t[0:128, : w3 // 2],
                                  in_=dram["xf6"][:, o3 // 2 : (o3 + w3) // 2])

            trigger_chunk(0)
            for name, t in [("wa3", wa3_t), ("ba3", ba3_t)]:
                nc.sync.dma_start(out=t[:], in_=dram[name][:])

            # PE HAM warm-up, gated on the first input chunk (via the copy
            # below): back-to-back full-128x128 matmuls give the activity
            # monitor a fully-busy 4096-cycle window, flipping the PE clock
            # gate from its default 1.2 GHz to 2.4 GHz; the main loop's own
            # dense stream keeps it warm from there.
            nc.vector.tensor_copy(out=wtmp[0:1, 0:1], in_=ds3_ts[0][0:1, 0:1])
            pw0 = pbp.tile([128, BLK], f32, tag="pb")
            for _ in range(N_WARMUP):
                nc.tensor.matmul(pw0[:, :], lhsT=wtmp[:, 0:128],
                                 rhs=wtmp[:, :], start=True, stop=True)

            for name, t in [("wh", wh_t), ("wxw", wxw_t), ("bb6", bb6_t)]:
                nc.sync.dma_start(out=t[:], in_=dram[name][:])
            trigger_chunk(1)
            for name, t in [("woutm", woutm_t), ("woutx", woutx_t),
                            ("bo6", bo6_t)]:
                nc.sync.dma_start(out=t[:], in_=dram[name][:])
            trigger_chunk(2)
            nc.sync.dma_start(out=recip_t[:], in_=dram["recip"][:])

            g1s = gpool.tile([124, G1], bf16, tag="g1s")
            g1m = gpool.tile([124, G1], bf16, tag="g1m")
            g2s = gpool.tile([124, c6p], f32, tag="g2s")
            g2m = gpool.tile([124, c6p], bf16, tag="g2m")
            g2sb = gpool.tile([124, c6p], bf16, tag="g2sb")
            out_t = outp.tile([96, c6p], f32, tag="out")
            nc.gpsimd.memset(g2s[:, :], 0.0)
            nc.gpsimd.memset(g2m[:, :], 0.0)

            lvl2_next = {k: 0 for k in classes}  # next slot to reduce
            final_done = set()
            chunk_cls = {
                cc: [k for k in classes
                     if c_k[k] < cc + BLK and c_k[k] + n48[k] > cc]
                for cc in range(0, c6p, BLK)
            }

            def _emit_lvl2(groups_ready):
                # incremental: reduce only the slots whose level-1 groups
                # completed, so each piece stays small and never head-of-line
                # blocks the DVE queue
                for k in classes:
                    nk = n48[k]
                    a = a_k[k]
                    done = lvl2_next[k]
                    if done >= nk:
                        continue
                    ready = min(nk, max(0, (groups_ready - a) // k))
                    if ready <= done:
                        continue
                    c0 = c_k[k]
                    gv_s = g1s[0:124, a + done * k : a + ready * k].rearrange(
                        "p (n k) -> p n k", k=k)
                    gv_m = g1m[0:124, a + done * k : a + ready * k].rearrange(
                        "p (n k) -> p n k", k=k)
                    nc.vector.tensor_reduce(
                        out=g2s[0:124, c0 + done : c0 + ready], in_=gv_s,
                        axis=AX, op=OP.add)
                    nc.vector.tensor_reduce(
                        out=g2m[0:124, c0 + done : c0 + ready], in_=gv_m,
                        axis=AX, op=OP.max)
                    lvl2_next[k] = ready

            def _maybe_final():
                # emit mean-scale + final GEMM + output DMA for any 512-col
                # chunk whose classes have all been level-2 reduced
                for cc in range(0, c6p, BLK):
                    if cc in final_done:
                        continue
                    if not all(lvl2_next[k] >= n48[k] for k in chunk_cls[cc]):
                        continue
                    final_done.add(cc)
                    nc.vector.tensor_mul(out=g2sb[:, cc : cc + BLK],
                                         in0=g2s[:, cc : cc + BLK],
                                         in1=recip_t[:, cc : cc + BLK])
                    po = pbp.tile([128, BLK], f32, tag="pb")
                    nc.tensor.matmul(
                        po[0:96, :], lhsT=woutm_t[:, :],
                        rhs=g2sb[0:124, cc : cc + BLK],
                        start=True, stop=False,
                    )
                    nc.tensor.matmul(
                        po[0:96, :], lhsT=woutx_t[:, :],
                        rhs=g2m[0:124, cc : cc + BLK],
                        start=False, stop=True,
                    )
                    nc.scalar.activation(out_t[0:96, cc : cc + BLK],
                                         po[0:96, :], RELU, bias=bo6_t[:, :])
                    nc.sync.dma_start(out=out_d[:, cc : cc + BLK],
                                      in_=out_t[0:96, cc : cc + BLK])

            for di, (o3, w3) in enumerate(chunk_list):
                o6, w6 = o3 // 2, w3 // 2
                ds3_t = ds3_ts[di % 3]
                xf6_t = xf6_ts[di % 3]
                for bl in range(w6 // BLK):
                    b = o6 // BLK + bl  # global pb-block index
                    # --- stage 1: 3-lane-packed MLPs ---
                    pa = pap.tile([128, 1024], f32, tag="pa")
                    for t in range(2):
                        nc.tensor.matmul(
                            pa[0:120, 512 * t : 512 * t + 512],
                            lhsT=wa3_t[:, :],
                            rhs=ds3_t[:, 1024 * bl + 512 * t
                                      : 1024 * bl + 512 * t + 512],
                            start=True, stop=True,
                        )
                    hx = hx_ts[b % 3]
                    nc.scalar.activation(hx[0:120, :], pa[0:120, :], RELU,
                                         bias=ba3_t[:, :])
                    # --- stage 2: y pre-activation, two blocks per pb pair ---
                    poff = b % 2
                    if poff == 0:
                        pb = pbp.tile([128, 2 * BLK], f32, tag="pb")
                    ph = pb[:, BLK * poff : BLK * poff + BLK]
                    nc.tensor.matmul(
                        ph[0:124, :], lhsT=wxw_t[:, :],
                        rhs=xf6_t[:, BLK * bl : BLK * bl + BLK],
                        start=True, stop=False,
                    )
                    nc.tensor.matmul(
                        ph[0:60, :], lhsT=wh_t[:, 0:60],
                        rhs=hx[:, 0:512],
                        start=False, stop=True, skip_group_check=True,
                    )
                    nc.tensor.matmul(
                        ph[64:124, :], lhsT=wh_t[:, 0:60],
                        rhs=hx[:, 512:1024],
                        start=False, stop=True, skip_group_check=True,
                    )
                    # --- reluB over the pair, into the octet y tile ---
                    ooff = b % 8
                    if ooff == 0:
                        q0 = b
                        yq = yp.tile([124, 8 * BLK], bf16, tag="yq")
                    if poff == 1 or b == nblk - 1:
                        w_pair = BLK * (poff + 1)
                        ysl = yq[:, BLK * (ooff - poff)
                                 : BLK * (ooff - poff) + w_pair]
                        if (b // 2) % RELUB_ACT_MOD == 1:
                            nc.scalar.activation(ysl, pb[0:124, :w_pair],
                                                 RELU, bias=bb6_t[:, :])
                        else:
                            nc.vector.tensor_scalar(
                                out=ysl, in0=pb[0:124, :w_pair],
                                scalar1=bb6_t[:, :], scalar2=0.0,
                                op0=OP.add, op1=OP.max)
                    # --- lvl-1 round 1 per half-octet (keeps the DVE queue
                    # from going bursty), rounds 2+3 batched per octet ---
                    if ooff == 3 and b != nblk - 1:
                        t1s = t1p.tile([124, 2048], bf16, tag="t1s")
                        t1m = m1p.tile([124, 2048], bf16, tag="t1m")
                        yv = yq[:, 0 : BLK * 4].rearrange(
                            "p (g k) -> p g k", k=8)
                        nc.vector.tensor_tensor(
                            out=t1s[:, 0:1024].rearrange(
                                "p (g k) -> p g k", k=4),
                            in0=yv[:, :, 0:4], in1=yv[:, :, 4:8], op=OP.add)
                        nc.vector.tensor_tensor(
                            out=t1m[:, 0:1024].rearrange(
                                "p (g k) -> p g k", k=4),
                            in0=yv[:, :, 0:4], in1=yv[:, :, 4:8], op=OP.max)
                    if ooff == 7 or b == nblk - 1:
                        nq = b - q0 + 1
                        nq1 = min(nq, 4)  # blocks already in round 1
                        if nq <= 4:
                            t1s = t1p.tile([124, 2048], bf16, tag="t1s")
                            t1m = m1p.tile([124, 2048], bf16, tag="t1m")
                            nq1 = 0
                        if nq > nq1:
                            yv = yq[:, BLK * nq1 : BLK * nq].rearrange(
                                "p (g k) -> p g k", k=8)
                            nc.vector.tensor_tensor(
                                out=t1s[:, 256 * nq1 : 256 * nq].rearrange(
                                    "p (g k) -> p g k", k=4),
                                in0=yv[:, :, 0:4], in1=yv[:, :, 4:8],
                                op=OP.add)
                            nc.vector.tensor_tensor(
                                out=t1m[:, 256 * nq1 : 256 * nq].rearrange(
                                    "p (g k) -> p g k", k=4),
                                in0=yv[:, :, 0:4], in1=yv[:, :, 4:8],
                                op=OP.max)
                        for t1_, g1_, op_ in ((t1s, g1s, OP.add),
                                              (t1m, g1m, OP.max)):
                            t1v = t1_[:, 0 : 256 * nq].rearrange(
                                "p (g k) -> p g k", k=4)
                            t2 = t2p.tile([124, 1024], bf16, tag="t2")
                            t2v = t2[:, 0 : 128 * nq].rearrange(
                                "p (g k) -> p g k", k=2)
                            nc.vector.tensor_tensor(
                                out=t2v, in0=t1v[:, :, 0:2],
                                in1=t1v[:, :, 2:4], op=op_)
                            nc.vector.tensor_tensor(
                                out=g1_[0:124, 64 * q0 : 64 * (q0 + nq)],
                                in0=t2v[:, :, 0], in1=t2v[:, :, 1], op=op_)
                        _emit_lvl2(64 * (q0 + nq))
                        _maybe_final()

                # prefetch: chunk di+3 reuses this chunk's tiles; emitting
                # the trigger after this chunk's readers gives it the right
                # WAR dependency while still running ~2 chunks ahead.
                if di + 3 < len(chunk_list):
                    trigger_chunk(di + 3)

            _emit_lvl2(G1 * 2)
            _maybe_final()
            assert len(final_done) == c6p // BLK

    nc.compile()
    return nc


# ----------------------------------------------------------------------------
# Entry point
# ----------------------------------------------------------------------------

def _gather_output(core_data, outs):
    OUT = np.zeros((C, D_OUT), dtype=np.float32)
    for ci in range(N_CORES):
        _, _, _, slot_comm = core_data[ci]
        oimg = np.asarray(outs[ci], dtype=np.float32)
        for lj in range(N_LANES):
            comms = slot_comm[lj]
            real = comms >= 0
            OUT[comms[real]] = oimg[16 * lj : 16 * lj + 16, : len(real)][:, real].T
    return OUT


def kernel(x, dataset_x, community, multi_community_nodes, multi_community_index,
           W_demo, b_demo, W_purch, b_purch, W_feat, b_feat, W_out, b_out,
           _run_device=None):
    x = np.asarray(x, dtype=np.float32)
    dataset_x = np.asarray(dataset_x, dtype=np.float32)
    community = np.asarray(community)
    multi_community_nodes = np.asarray(multi_community_nodes)
    multi_community_index = np.asarray(multi_community_index)
    params = tuple(
        np.asarray(p, dtype=np.float32)
        for p in (W_demo, b_demo, W_purch, b_purch, W_feat, b_feat, W_out, b_out)
    )

    core_data, layout = _plan(community, multi_community_index,
                              multi_community_nodes)
    shared = _build_shared_inputs(params)
    in_maps = []
    for ci in range(N_CORES):
        m = _build_core_inputs(core_data[ci], layout, x, dataset_x)
        m.update(shared)
        in_maps.append(m)

    if _run_device is None:
        from concourse.bass_utils import run_bass_kernel_spmd

        nc = _build_nc(layout)
        res = run_bass_kernel_spmd(nc, in_maps, list(range(N_CORES)))
        outs = [res.results[i]["out"] for i in range(N_CORES)]
    else:
        outs = _run_device(layout, in_maps)

    return _gather_output(core_data, outs)


# revision 60
# speedup vs baseline: 1.2439x; 1.0562x over previous
"""DeepWalk community-pooling kernel for 8 trn2 NeuronCores (v2).

Pipeline (per core, SPMD identical program, per-core data):
  host: sort extended rows (N + multi duplicates) by community, pad each
        community to a multiple of 8 rows, deal communities per size-class
        round-robin onto 48 (core, lane) slots (6 lanes/core) so every
        slot has an identical class profile.
  device, per 512-column "pb block" (512 stream indices x 6 lanes = 3072
  rows):
    mmA  : ds3^T 3-lane-packed [60,1024] x wa3 -> pa [120,1024] psum
    hx   : ACT relu+bias -> hx3 bf16 [120,1024]
    mm_h : wh^T x hx3 chunks -> pb[0:60] / pb[64:124] (accumulate)
    mm_xw: wxw^T x xf6 [126,512] 6-lane-packed -> pb (x-contribution +
           pad-flag), one matmul at 6-row/col density
    reluB: (pb + b_feat) relu -> y bf16 [124,512]  (ACT or DVE, balanced)
    sum  : DVE TT-tree radix-8 (2x bf16 mode) -> g1s
    max  : GPSIMD TT-tree radix-8 -> g1m
    lvl2 : per size-class tensor_reduce over k groups -> g2s (f32), g2m
  tail:  mean = g2s * recip (host-provided reciprocals), final GEMM
         relu(W_out^T [mean; max] + b_out) -> out [96, c6p]
  host: gather per-lane outputs back to the global community order.
"""

import sys

import numpy as np

sys.path.insert(0, "/opt/trn_rl_repo")

import ml_dtypes  # noqa: E402

BF16 = ml_dtypes.bfloat16
FP8 = ml_dtypes.float8_e4m3fn

N = 2_000_000
M = 500_000
C = 50_000
D_OUT = 16
N_CORES = 8
N_LANES = 6  # per core
SLOTS = N_CORES * N_LANES
BLK = 512  # pb columns per block
FLAG_PAD = -32768.0
W3_DMA = 8192  # ds3 cols per input DMA tile (= 4096 stream idx)
LANE_OFF = [0, 20, 40, 64, 84, 104]  # partition offset of each lane block
RELUB_ACT_MOD = 4  # pair p with p % MOD == 1 does reluB on ACT, rest DVE
N_WARMUP = 17  # back-to-back warm-up matmuls to flip the PE HAM to 2.4 GHz


# ----------------------------------------------------------------------------
# Host-side planning
# ----------------------------------------------------------------------------

def _plan(community, multi_community_index, multi_community_nodes):
    """Sort/pad/shard rows. Returns per-core row sources + static layout."""
    seg = np.concatenate([community, multi_community_index]).astype(np.int64)
    src = np.concatenate(
        [np.arange(N, dtype=np.int64), multi_community_nodes.astype(np.int64)]
    )

    counts = np.bincount(seg, minlength=C)
    kcls = np.maximum((counts + 7) // 8, 1).astype(np.int64)  # class = #groups
    assert kcls.max() <= 64, f"community too large: {counts.max()} rows"

    order = np.argsort(seg, kind="stable")
    src_sorted = src[order]
    starts = np.zeros(C + 1, dtype=np.int64)
    np.cumsum(counts, out=starts[1:])

    # communities per class, dealt round-robin to 48 (core,lane) slots.
    # Classes are laid out largest-first so the level-2 reductions of the
    # big classes complete early and the final GEMM pipelines with the
    # main loop.
    classes = np.unique(kcls)[::-1]
    slot_comms = [[[] for _ in range(N_LANES)] for _ in range(N_CORES)]
    n48 = {}  # class k -> communities per slot
    for k in classes:
        comms = np.nonzero(kcls == k)[0]
        n48[int(k)] = (len(comms) + SLOTS - 1) // SLOTS
        for i, g in enumerate(comms):
            s = i % SLOTS
            slot_comms[s // N_LANES][s % N_LANES].append(int(g))
    classes = [int(k) for k in classes]

    # per-lane group/community layout (identical across all cores/lanes)
    lane_groups = sum(n48[k] * k for k in classes)
    c6 = sum(n48[k] for k in classes)  # community slots per lane
    c6p = ((c6 + BLK - 1) // BLK) * BLK
    lane_rows = lane_groups * 8
    lane_len = ((lane_rows + BLK - 1) // BLK) * BLK

    # class offsets (group units and community-slot units)
    a_k, c_k, ga, ca = {}, {}, 0, 0
    for k in classes:
        a_k[k] = ga
        c_k[k] = ca
        ga += n48[k] * k
        ca += n48[k]

    # per (core,lane): row source indices (-1 = padding), per-slot counts
    core_data = []
    for ci in range(N_CORES):
        lane_src = np.full((N_LANES, lane_len), -1, dtype=np.int64)
        lane_flag = np.full((N_LANES, lane_len), FLAG_PAD, dtype=np.float32)
        slot_count = np.zeros((N_LANES, c6p), dtype=np.int64)
        slot_comm = np.full((N_LANES, c6p), -1, dtype=np.int64)
        for lj in range(N_LANES):
            comms = slot_comms[ci][lj]
            by_k = {k: [] for k in classes}
            for g in comms:
                by_k[int(kcls[g])].append(g)
            pos = 0
            for k in classes:
                lst = by_k[k]
                for i in range(n48[k]):
                    slot = c_k[k] + i
                    if i < len(lst):
                        g = lst[i]
                        cnt = int(counts[g])
                        s0 = starts[g]
                        lane_src[lj, pos : pos + cnt] = src_sorted[s0 : s0 + cnt]
                        lane_flag[lj, pos : pos + cnt] = 0.0
                        slot_count[lj, slot] = cnt
                        slot_comm[lj, slot] = g
                    pos += 8 * k
            assert pos == lane_rows
        core_data.append((lane_src, lane_flag, slot_count, slot_comm))

    layout = dict(
        classes=classes, n48=n48, a_k=a_k, c_k=c_k,
        c6=c6, c6p=c6p, lane_len=lane_len, lane_groups=lane_groups,
    )
    return core_data, layout


def _build_core_inputs(core_dat, layout, x, dataset_x):
    """Build the DRAM images for one core."""
    lane_src, lane_flag, slot_count, _ = core_dat
    lane_len = layout["lane_len"]
    c6p = layout["c6p"]
    nblk = lane_len // BLK
    F3 = 2 * lane_len
    F6 = lane_len

    idx = np.maximum(lane_src, 0)

    # ds3 [60, F3]: col 1024b+512t+j holds lanes {3t,3t+1,3t+2} at stream
    # index 512b+j; lane 3t+m occupies partitions 20m..20m+20. fp8: the
    # demo/purch MLP path tolerates e4m3 (verified ~0.004 end-to-end).
    arr = dataset_x[idx].astype(FP8)               # [6, lane_len, 20]
    arrv = arr.reshape(2, 3, nblk, BLK, 20)        # [t, m, b, j, f]
    ds3 = np.zeros((64, F3), dtype=FP8)
    ds3[0:60] = arrv.transpose(1, 4, 2, 0, 3).reshape(60, F3)

    # xf6 [126, F6]: col i holds all 6 lanes at stream index i;
    # lane l occupies partitions 21l..21l+20 (+ flag channel at 21l+20).
    xv = x[idx].astype(BF16)                       # [6, lane_len, 20]
    xf6 = np.zeros((128, F6), dtype=BF16)
    for l in range(N_LANES):
        xf6[21 * l : 21 * l + 20] = xv[l].T
        xf6[21 * l + 20] = lane_flag[l].astype(BF16)

    recip = np.ones((124, c6p), dtype=np.float32)
    for l in range(N_LANES):
        r = 1.0 / np.maximum(slot_count[l], 1).astype(np.float32)
        off = LANE_OFF[l]
        recip[off : off + 20, :] = r[None, :]

    return dict(ds3=ds3, xf6=xf6, recip=recip)


def _build_shared_inputs(params):
    (W_demo, b_demo, W_purch, b_purch, W_feat, b_feat, W_out, b_out) = params

    # mmA stationary [128, 120]: 3 lanes; lane t ds feats at partitions
    # 20t..20t+20 -> h (demo|purch) at out cols 40t..40t+40. All matmul
    # contracts are zero-padded to the full 128 rows: the PE HAM activity
    # monitor only un-throttles the clock gate (1.2 -> 2.4 GHz) for
    # full-height operands.
    wa3 = np.zeros((128, 120), dtype=FP8)
    for t in range(3):
        wa3[20 * t : 20 * t + 8, 40 * t : 40 * t + 20] = W_demo
        wa3[20 * t + 8 : 20 * t + 20, 40 * t + 20 : 40 * t + 40] = W_purch

    ba3 = np.zeros((120, 1), dtype=np.float32)
    for t in range(3):
        ba3[40 * t : 40 * t + 20, 0] = b_demo
        ba3[40 * t + 20 : 40 * t + 40, 0] = b_purch

    # mm_h stationary [128, 60]: lane t h-feats at 40t..40t+40 -> y cols
    # 20t..20t+20 (chunk A lanes 0-2 at pb[0:60], chunk B lanes 3-5 at
    # pb[64:124])
    wh = np.zeros((128, 60), dtype=BF16)
    for t in range(3):
        wh[40 * t : 40 * t + 40, 20 * t : 20 * t + 20] = W_feat[0:40]

    # mm_xw stationary [128, 124]: 6-lane-packed x -> x-part of y, plus the
    # pad flag channel -> -32768 on that lane's 20 y cols
    wxw = np.zeros((128, 124), dtype=BF16)
    for l in range(N_LANES):
        off = LANE_OFF[l]
        wxw[21 * l : 21 * l + 20, off : off + 20] = W_feat[40:60]
        wxw[21 * l + 20, off : off + 20] = 1.0

    bb6 = np.zeros((124, 1), dtype=np.float32)
    for l in range(N_LANES):
        off = LANE_OFF[l]
        bb6[off : off + 20, 0] = b_feat

    # final GEMM stationaries [124, 96]: lane l mean/max rows -> out cols
    # 16l..16l+16
    woutm = np.zeros((124, 96), dtype=BF16)
    woutx = np.zeros((124, 96), dtype=BF16)
    for l in range(N_LANES):
        off = LANE_OFF[l]
        woutm[off : off + 20, 16 * l : 16 * l + 16] = W_out[0:20]
        woutx[off : off + 20, 16 * l : 16 * l + 16] = W_out[20:40]

    bo6 = np.zeros((96, 1), dtype=np.float32)
    for l in range(N_LANES):
        bo6[16 * l : 16 * l + 16, 0] = b_out

    return dict(wa3=wa3, ba3=ba3, wh=wh, wxw=wxw, bb6=bb6,
                woutm=woutm, woutx=woutx, bo6=bo6)


# ----------------------------------------------------------------------------
# Device kernel
# ----------------------------------------------------------------------------

def _build_nc(layout):
    import concourse.bacc as bacc
    import concourse.mybir as mybir
    from concourse import tile

    f32 = mybir.dt.float32
    bf16 = mybir.dt.bfloat16
    f8 = mybir.dt.float8e4

    lane_len = layout["lane_len"]
    c6p = layout["c6p"]
    nblk = lane_len // BLK
    F3 = 2 * lane_len
    F6 = lane_len
    G1 = nblk * 64  # lvl-1 group columns (64 per block)
    classes = layout["classes"]
    n48 = layout["n48"]
    a_k = layout["a_k"]
    c_k = layout["c_k"]

    nc = bacc.Bacc("TRN2", target_bir_lowering=False, debug=False)

    dt_map = dict(ds3=f8, xf6=bf16, recip=f32, wa3=f8, wh=bf16, wxw=bf16,
                  woutm=bf16, woutx=bf16, ba3=f32, bb6=f32, bo6=f32)
    shapes = dict(ds3=[64, F3], xf6=[128, F6], recip=[124, c6p],
                  wa3=[128, 120], wh=[128, 60], wxw=[128, 124],
                  woutm=[124, 96], woutx=[124, 96],
                  ba3=[120, 1], bb6=[124, 1], bo6=[96, 1])
    dram = {
        name: nc.declare_dram_parameter(name, shapes[name], dt_map[name],
                                        isOutput=False)
        for name in shapes
    }
    out_d = nc.declare_dram_parameter("out", [96, c6p], f32, isOutput=True)

    AX = mybir.AxisListType.X
    OP = mybir.AluOpType
    RELU = mybir.ActivationFunctionType.Relu

    with tile.TileContext(nc) as tc:
        with (
            tc.tile_pool(name="wpool", bufs=1) as wpool,
            tc.tile_pool(name="g", bufs=1) as gpool,
            tc.tile_pool(name="ds3p", bufs=2) as ds3p,
            tc.tile_pool(name="xf6p", bufs=2) as xf6p,
            tc.tile_pool(name="hxp", bufs=3) as hxp,
            tc.tile_pool(name="yp", bufs=3) as yp,
            tc.tile_pool(name="t1p", bufs=2) as t1p,
            tc.tile_pool(name="t2p", bufs=4) as t2p,
            tc.tile_pool(name="m1p", bufs=2) as m1p,
            tc.tile_pool(name="pa", bufs=2, space="PSUM") as pap,
            tc.tile_pool(name="pb", bufs=2, space="PSUM") as pbp,
            tc.tile_pool(name="outp", bufs=1) as outp,
        ):
            wa3_t = wpool.tile([128, 120], f8, tag="wa3")
            wh_t = wpool.tile([128, 60], bf16, tag="wh")
            wxw_t = wpool.tile([128, 124], bf16, tag="wxw")
            woutm_t = wpool.tile([124, 96], bf16, tag="woutm")
            woutx_t = wpool.tile([124, 96], bf16, tag="woutx")
            ba3_t = wpool.tile([120, 1], f32, tag="ba3")
            bb6_t = wpool.tile([124, 1], f32, tag="bb6")
            bo6_t = wpool.tile([96, 1], f32, tag="bo6")
            recip_t = wpool.tile([124, c6p], f32, tag="recip")

            # fixed hand-rotated input/hx tiles, zero-padded to 128 rows so
            # every matmul streams a full-height rhs (HAM activity)
            ds3_ts = [wpool.tile([128, W3_DMA], f8, tag=f"ds3{i}",
                                 name=f"ds3{i}") for i in range(3)]
            xf6_ts = [wpool.tile([128, W3_DMA // 2], bf16, tag=f"xf6{i}",
                                 name=f"xf6{i}") for i in range(3)]
            hx_ts = [wpool.tile([128, 1024], bf16, tag=f"hx{i}",
                                name=f"hx{i}") for i in range(3)]
            wtmp = wpool.tile([128, 512], bf16, tag="wtmp")
            nc.gpsimd.memset(wtmp[:, :], 0.0)
            # the first 4096 cols gate chunk 0's matmuls; the rest of tile 0
            # is only read from chunk 3 on, so zero it off the critical path
            nc.gpsimd.memset(ds3_ts[0][64:128, 0:4096], 0.0)
            nc.gpsimd.memset(ds3_ts[0][64:128, 4096:W3_DMA], 0.0)
            nc.gpsimd.memset(ds3_ts[1][64:128, :], 0.0)
            nc.vector.memset(ds3_ts[2][64:128, :], 0.0)
            nc.scalar.memzero(hx_ts[0][96:128, :])
            nc.scalar.memzero(hx_ts[1][96:128, :])
            nc.scalar.memzero(hx_ts[2][96:128, :])

            # input chunk schedule: two small leading chunks cut the
            # time-to-first-block; input triggers precede the bulky weight
            # transfers on the sync queue.
            chunk_list = []
            o3 = 0
            while o3 < F3:
                w3 = min(4096 if o3 < 8192 else W3_DMA, F3 - o3)
                chunk_list.append((o3, w3))
                o3 += w3

            def trigger_chunk(di):
                o3, w3 = chunk_list[di]
                ds3_t = ds3_ts[di % 3]
                xf6_t = xf6_ts[di % 3]
                nc.sync.dma_start(out=ds3_t[0:64, :w3],
                                  in_=dram["ds3"][:, o3 : o3 + w3])
                nc.sync.dma_start(out=xf6_t[0:128, : w3 // 2],
                                  in_=dram["xf6"][:, o3 // 2 : (o3 + w3) // 2])

            trigger_chunk(0)
            for name, t in [("wa3", wa3_t), ("ba3", ba3_t)]:
                nc.sync.dma_start(out=t[:], in_=dram[name][:])

            # PE HAM warm-up, gated on the first input chunk (via the copy
            # below): back-to-back full-128x128 matmuls give the activity
            # monitor a fully-busy 4096-cycle window, flipping the PE clock
            # gate from its default 1.2 GHz to 2.4 GHz; the main loop's own
            # dense stream keeps it warm from there.
            nc.vector.tensor_copy(out=wtmp[0:1, 0:1], in_=ds3_ts[0][0:1, 0:1])
            pw0 = pbp.tile([128, BLK], f32, tag="pb")
            for _ in range(N_WARMUP):
                nc.tensor.matmul(pw0[:, :], lhsT=wtmp[:, 0:128],
                                 rhs=wtmp[:, :], start=True, stop=True)

            for name, t in [("wh", wh_t), ("wxw", wxw_t), ("bb6", bb6_t)]:
                nc.sync.dma_start(out=t[:], in_=dram[name][:])
            trigger_chunk(1)
            for name, t in [("woutm", woutm_t), ("woutx", woutx_t),
                            ("bo6", bo6_t)]:
                nc.sync.dma_start(out=t[:], in_=dram[name][:])
            trigger_chunk(2)

            g1s = gpool.tile([124, G1], bf16, tag="g1s")
            g1m = gpool.tile([124, G1], bf16, tag="g1m")
            g2s = gpool.tile([124, c6p], f32, tag="g2s")
            g2m = gpool.tile([124, c6p], bf16, tag="g2m")
            g2sb = gpool.tile([124, c6p], bf16, tag="g2sb")
            out_t = outp.tile([96, c6p], f32, tag="out")
            nc.gpsimd.memset(g2s[:, :], 0.0)
            nc.gpsimd.memset(g2m[:, :], 0.0)

            lvl2_next = {k: 0 for k in classes}  # next slot to reduce
            final_done = set()
            chunk_cls = {
                cc: [k for k in classes
                     if c_k[k] < cc + BLK and c_k[k] + n48[k] > cc]
                for cc in range(0, c6p, BLK)
            }

            def _emit_lvl2(groups_ready):
                # incremental: reduce only the slots whose level-1 groups
                # completed, so each piece stays small and never head-of-line
                # blocks the DVE queue
                for k in classes:
                    nk = n48[k]
                    a = a_k[k]
                    done = lvl2_next[k]
                    if done >= nk:
                        continue
                    ready = min(nk, max(0, (groups_ready - a) // k))
                    if ready <= done:
                        continue
                    c0 = c_k[k]
                    gv_s = g1s[0:124, a + done * k : a + ready * k].rearrange(
                        "p (n k) -> p n k", k=k)
                    gv_m = g1m[0:124, a + done * k : a + ready * k].rearrange(
                        "p (n k) -> p n k", k=k)
                    nc.vector.tensor_reduce(
                        out=g2s[0:124, c0 + done : c0 + ready], in_=gv_s,
                        axis=AX, op=OP.add)
                    nc.vector.tensor_reduce(
                        out=g2m[0:124, c0 + done : c0 + ready], in_=gv_m,
                        axis=AX, op=OP.max)
                    lvl2_next[k] = ready

            def _maybe_final():
                # emit mean-scale + final GEMM + output DMA for any 512-col
                # chunk whose classes have all been level-2 reduced
                for cc in range(0, c6p, BLK):
                    if cc in final_done:
                        continue
                    if not all(lvl2_next[k] >= n48[k] for k in chunk_cls[cc]):
                        continue
                    final_done.add(cc)
                    nc.vector.tensor_mul(out=g2sb[:, cc : cc + BLK],
                                         in0=g2s[:, cc : cc + BLK],
                                         in1=recip_t[:, cc : cc + BLK])
                    po = pbp.tile([128, BLK], f32, tag="pb")
                    nc.tensor.matmul(
                        po[0:96, :], lhsT=woutm_t[:, :],
                        rhs=g2sb[0:124, cc : cc + BLK],
                        start=True, stop=False,
                    )
                    nc.tensor.matmul(
                        po[0:96, :], lhsT=woutx_t[:, :],
                        rhs=g2m[0:124, cc : cc + BLK],
                        start=False, stop=True,
                    )
                    nc.scalar.activation(out_t[0:96, cc : cc + BLK],
                                         po[0:96, :], RELU, bias=bo6_t[:, :])
                    nc.sync.dma_start(out=out_d[:, cc : cc + BLK],
                                      in_=out_t[0:96, cc : cc + BLK])

            for di, (o3, w3) in enumerate(chunk_list):
                o6, w6 = o3 // 2, w3 // 2
                ds3_t = ds3_ts[di % 3]
                xf6_t = xf6_ts[di % 3]
                for bl in range(w6 // BLK):
                    b = o6 // BLK + bl  # global pb-block index
                    # --- stage 1: 3-lane-packed MLPs ---
                    pa = pap.tile([128, 1024], f32, tag="pa")
                    for t in range(2):
                        nc.tensor.matmul(
                            pa[0:120, 512 * t : 512 * t + 512],
                            lhsT=wa3_t[:, :],
                            rhs=ds3_t[:, 1024 * bl + 512 * t
                                      : 1024 * bl + 512 * t + 512],
                            start=True, stop=True,
                        )
                    hx = hx_ts[b % 3]
                    nc.scalar.activation(hx[0:120, :], pa[0:120, :], RELU,
                                         bias=ba3_t[:, :])
                    # --- stage 2: y pre-activation, two blocks per pb pair ---
                    poff = b % 2
                    if poff == 0:
                        pb = pbp.tile([128, 2 * BLK], f32, tag="pb")
                    ph = pb[:, BLK * poff : BLK * poff + BLK]
                    nc.tensor.matmul(
                        ph[0:124, :], lhsT=wxw_t[:, :],
                        rhs=xf6_t[:, BLK * bl : BLK * bl + BLK],
                        start=True, stop=False,
                    )
                    nc.tensor.matmul(
                        ph[0:60, :], lhsT=wh_t[:, 0:60],
                        rhs=hx[:, 0:512],
                        start=False, stop=True, skip_group_check=True,
                    )
                    nc.tensor.matmul(
                        ph[64:124, :], lhsT=wh_t[:, 0:60],
                        rhs=hx[:, 512:1024],
                        start=False, stop=True, skip_group_check=True,
                    )
                    # --- reluB over the pair, into the octet y tile ---
                    ooff = b % 8
                    if ooff == 0:
                        q0 = b
                        yq = yp.tile([124, 8 * BLK], bf16, tag="yq")
                    if poff == 1 or b == nblk - 1:
                        w_pair = BLK * (poff + 1)
                        ysl = yq[:, BLK * (ooff - poff)
                                 : BLK * (ooff - poff) + w_pair]
                        if (b // 2) % RELUB_ACT_MOD == 1:
                            nc.scalar.activation(ysl, pb[0:124, :w_pair],
                                                 RELU, bias=bb6_t[:, :])
                        else:
                            nc.vector.tensor_scalar(
                                out=ysl, in0=pb[0:124, :w_pair],
                                scalar1=bb6_t[:, :], scalar2=0.0,
                                op0=OP.add, op1=OP.max)
                    # --- lvl-1 round 1 per half-octet (keeps the DVE queue
                    # from going bursty), rounds 2+3 batched per octet ---
                    if ooff == 3 and b != nblk - 1:
                        t1s = t1p.tile([124, 2048], bf16, tag="t1s")
                        t1m = m1p.tile([124, 2048], bf16, tag="t1m")
                        yv = yq[:, 0 : BLK * 4].rearrange(
                            "p (g k) -> p g k", k=8)
                        nc.vector.tensor_tensor(
                            out=t1s[:, 0:1024].rearrange(
                                "p (g k) -> p g k", k=4),
                            in0=yv[:, :, 0:4], in1=yv[:, :, 4:8], op=OP.add)
                        nc.vector.tensor_tensor(
                            out=t1m[:, 0:1024].rearrange(
                                "p (g k) -> p g k", k=4),
                            in0=yv[:, :, 0:4], in1=yv[:, :, 4:8], op=OP.max)
                    if ooff == 7 or b == nblk - 1:
                        nq = b - q0 + 1
                        nq1 = min(nq, 4)  # blocks already in round 1
                        if nq <= 4:
                            t1s = t1p.tile([124, 2048], bf16, tag="t1s")
                            t1m = m1p.tile([124, 2048], bf16, tag="t1m")
                            nq1 = 0
                        if nq > nq1:
                            yv = yq[:, BLK * nq1 : BLK * nq].rearrange(
                                "p (g k) -> p g k", k=8)
                            nc.vector.tensor_tensor(
                                out=t1s[:, 256 * nq1 : 256 * nq].rearrange(
                                    "p (g k) -> p g k", k=4),
                                in0=yv[:, :, 0:4], in1=yv[:, :, 4:8],
                                op=OP.add)
                            nc.vector.tensor_tensor(
                                out=t1m[:, 256 * nq1 : 256 * nq].rearrange(
                                    "p (g k) -> p g k", k=4),
                                in0=yv[:, :, 0:4], in1=yv[:, :, 4:8],
                                op=OP.max)
                        for t1_, g1_, op_ in ((t1s, g1s, OP.add),
                                              (t1m, g1m, OP.max)):
                            t1v = t1_[:, 0 : 256 * nq].rearrange(
                                "p (g k) -> p g k", k=4)
                            t2 = t2p.tile([124, 1024], bf16, tag="t2")
                            t2v = t2[:, 0 : 128 * nq].rearrange(
                                "p (g k) -> p g k", k=2)
                            nc.vector.tensor_tensor(
                                out=t2v, in0=t1v[:, :, 0:2],
                                in1=t1v[:, :, 2:4], op=op_)
                            nc.vector.tensor_tensor(
                                out=g1_[0:124, 64 * q0 : 64 * (q0 + nq)],
                                in0=t2v[:, :, 0], in1=t2v[:, :, 1], op=op_)
                        _emit_lvl2(64 * (q0 + nq))
                        _maybe_final()

                # prefetch: chunk di+3 reuses this chunk's tiles; emitting
                # the trigger after this chunk's readers gives it the right
                # WAR dependency while still running ~2 chunks ahead.
                if di + 3 < len(chunk_list):
                    trigger_chunk(di + 3)
                if di == 1:
                    # recip is first read by the mid-kernel finals; keep its
                    # 0.77 MB transfer out of the startup-critical window
                    nc.sync.dma_start(out=recip_t[:], in_=dram["recip"][:])

            _emit_lvl2(G1 * 2)
            _maybe_final()
            assert len(final_done) == c6p // BLK

    nc.compile()
    return nc


# ----------------------------------------------------------------------------
# Entry point
# ----------------------------------------------------------------------------

def _gather_output(core_data, outs):
    OUT = np.zeros((C, D_OUT), dtype=np.float32)
    for ci in range(N_CORES):
        _, _, _, slot_comm = core_data[ci]
        oimg = np.asarray(outs[ci], dtype=np.float32)
        for lj in range(N_LANES):
            comms = slot_comm[lj]
            real = comms >= 0
            OUT[comms[real]] = oimg[16 * lj : 16 * lj + 16, : len(real)][:, real].T
    return OUT


def kernel(x, dataset_x, community, multi_community_nodes, multi_community_index,
           W_demo, b_demo, W_purch, b_purch, W_feat, b_feat, W_out, b_out,
           _run_device=None):
    x = np.asarray(x, dtype=np.float32)
    dataset_x = np.asarray(dataset_x, dtype=np.float32)
    community = np.asarray(community)
    multi_community_nodes = np.asarray(multi_community_nodes)
    multi_community_index = np.asarray(multi_community_index)
    params = tuple(
        np.asarray(p, dtype=np.float32)
        for p in (W_demo, b_demo, W_purch, b_purch, W_feat, b_feat, W_out, b_out)
    )

    core_data, layout = _plan(community, multi_community_index,
                              multi_community_nodes)
    shared = _build_shared_inputs(params)
    in_maps = []
    for ci in range(N_CORES):
        m = _build_core_inputs(core_data[ci], layout, x, dataset_x)
        m.update(shared)
        in_maps.append(m)

    if _run_device is None:
        from concourse.bass_utils import run_bass_kernel_spmd

        nc = _build_nc(layout)
        res = run_bass_kernel_spmd(nc, in_maps, list(range(N_CORES)))
        outs = [res.results[i]["out"] for i in range(N_CORES)]
    else:
        outs = _run_device(layout, in_maps)

    return _gather_output(core_data, outs)
